# revision 16
# baseline (speedup 1.0000x reference)
"""Bass/Tile TRN2 kernel: 16-head MHA (B=2, T=2048, D=1024, H=64) on 8 NeuronCores.

Sharding: 8-way tensor parallel over heads — core c handles heads {2c, 2c+1}
for BOTH batches. After attention each (batch, tq-block) "block" s (8 total)
is re-sharded so that core c owns COLUMN SLICE c (64 rows) of every block:
one small AllToAll per block ([8*128, 64] f16, 128 KB) fires as soon as that
block is normalized on all cores, fully overlapped with the remaining
attention compute. The output projection runs per block-PAIR (two 64-row
slices stacked into a full 128-partition matmul), also overlapped; only the
last block's AllToAll + projection remain on the tail.

Per-core device pipeline (all FLOPs on device):
  - Activation loads are chunked (xv/xk by d-chunk, xq by (tq-block, d-chunk))
    so the V projection starts on chunk 0 instead of after the full 12 MB
    preamble; Q projection is interleaved per tq-block into the attention
    loop. b=1 activations stream during b=0 attention.
  - QKV projections as f16 matmuls accumulating fp32 in PSUM; activations
    arrive pre-transposed ([D, T]) so the contraction dim d sits on SBUF
    partitions. 1/sqrt(H) is folded into Wq/bq on host.
  - Scores S^T[tk, tq] = K^T.T @ Q^T per head; the two heads are issued
    back-to-back as row-tiled (K=64, partitions 0-63 / 64-127) matmuls so they
    run concurrently on the PE array.
  - exp on ScalarE straight out of PSUM (3-bank [128,1536] tiles), bf16 out.
  - PV matmul with a ones-augmented V (65 stationary columns) so row 64 of the
    PV accumulator is the softmax denominator for free.
  - Normalize: DVE reciprocal of the denominator row straight out of PSUM,
    GPSIMD partition-broadcast across the 64 head-dim partitions, DVE multiply
    (no DRAM bounces).
  - Per-block AllToAll (see above), per-pair output projection, direct DMA of
    the [2*64, 512] result slabs to the output tensor.
Host does layout-only prep (transpose, f16 cast, weight slicing/replication)
and scatters the 8 cores' [8, 64, D] slabs into the full output.
"""

import os
import sys
from contextlib import ExitStack

import numpy as np

sys.path.insert(0, "/opt/trn_rl_repo")

import ml_dtypes  # noqa: E402

F16 = np.float16

B, T, D = 2, 2048, 1024
N_HEADS, H = 16, 64
NCORES = 8
GROUPS = [[0, 1, 2, 3, 4, 5, 6, 7]]
NLOC = 2            # heads per core
TQB = 512           # tq block size
NTQB = T // TQB     # 4
NBLK = B * NTQB     # 8 blocks total
TKC = 128           # tk chunk size
NTKC = T // TKC     # 16
DC = 128            # d chunk
NDC = D // DC       # 8
SLOTS = 2           # score-psum slots per exp tile ([128, 2*512] = 2 banks)
VA = 128            # V_aug stationary width: [V(64) | ones(1) | junk(63)]
NW = NLOC * H       # 128: projection output width per core
XO = 64             # output rows per core per block (column slice)
NHC = N_HEADS * H // 128  # 8 head-dim chunks in the output projection

_CACHE = {}


def _legalize_waits(bir_bytes):
    """This toolchain's walrus accepts at most ONE semaphore wait per
    instruction ("Too many sync wait commands"). Tile's sem assignment emits
    several. Hoist all but one wait of each instruction onto same-engine NoOps
    inserted immediately before it (engines execute their stream in order, so
    waiting earlier on the same engine is equivalent)."""
    import json

    j = json.loads(bir_bytes)
    ctr = 0
    for fn in j["functions"]:
        for blk in fn["blocks"]:
            out = []
            for ins in blk["instructions"]:
                si = ins.get("sync_info")
                waits = (si or {}).get("on_wait") or []
                if len(waits) > 1:
                    for w in waits[:-1]:
                        ctr += 1
                        out.append(
                            {
                                "engine": ins["engine"],
                                "ins": [],
                                "outs": [],
                                "name": f"waitfix-{ctr}",
                                "opcode": "NoOp",
                                "sync_info": {"on_wait": [w], "on_update": []},
                            }
                        )
                    si["on_wait"] = [waits[-1]]
                out.append(ins)
            blk["instructions"] = out
    return json.dumps(j).encode()


def _build():
    import concourse.bass as bass
    import concourse.mybir as mybir
    import concourse.tile as tile

    f32 = mybir.dt.float32
    f16 = mybir.dt.float16
    AF = mybir.ActivationFunctionType
    ALU = mybir.AluOpType

    nc = bass.Bass(
        "TRN2", target_bir_lowering=False, debug=False, num_devices=NCORES
    )

    qT = [nc.dram_tensor(f"qT{b}", [D, T], f16, kind="ExternalInput") for b in range(B)]
    kT = [nc.dram_tensor(f"kT{b}", [D, T], f16, kind="ExternalInput") for b in range(B)]
    vT = [nc.dram_tensor(f"vT{b}", [D, T], f16, kind="ExternalInput") for b in range(B)]
    wq = nc.dram_tensor("wq", [D, NW], f16, kind="ExternalInput")
    wk = nc.dram_tensor("wk", [D, NW], f16, kind="ExternalInput")
    wv = nc.dram_tensor("wv", [D, NW], f16, kind="ExternalInput")
    wp = nc.dram_tensor("wp", [N_HEADS * H, D], f16, kind="ExternalInput")
    bq = nc.dram_tensor("bq", [128, 1], f32, kind="ExternalInput")
    bk = nc.dram_tensor("bk", [128, 1], f32, kind="ExternalInput")
    bv = nc.dram_tensor("bv", [128, 1], f32, kind="ExternalInput")
    bp = nc.dram_tensor("bp", [128, D], f32, kind="ExternalInput")
    ident = nc.dram_tensor("ident", [128, 128], f16, kind="ExternalInput")
    out = nc.dram_tensor("out", [NBLK, XO, D], f32, kind="ExternalOutput")

    with tile.TileContext(nc) as tc, ExitStack() as ctx:
        p_const = ctx.enter_context(tc.tile_pool(name="const", bufs=1))
        p_xt = ctx.enter_context(tc.tile_pool(name="xt", bufs=1))
        p_qk = ctx.enter_context(tc.tile_pool(name="qk", bufs=4))
        p_va = ctx.enter_context(tc.tile_pool(name="va", bufs=2))
        p_pt = ctx.enter_context(tc.tile_pool(name="pt", bufs=3))
        p_a = ctx.enter_context(tc.tile_pool(name="a", bufs=3))
        p_at = ctx.enter_context(tc.tile_pool(name="at", bufs=2))
        p_o = ctx.enter_context(tc.tile_pool(name="o", bufs=2))
        p_dram = ctx.enter_context(tc.tile_pool(name="dram", bufs=1, space="DRAM"))

        # ---- small constant loads (wp/bp deferred past the x0 loads) --------
        wq_sb = p_const.tile([128, NDC * NW], f16)
        wk_sb = p_const.tile([128, NDC * NW], f16)
        wv_sb = p_const.tile([128, NDC * NW], f16)
        bq_sb = p_const.tile([128, 1], f32)
        bk_sb = p_const.tile([128, 1], f32)
        bv_sb = p_const.tile([128, 1], f32)
        id_sb = p_const.tile([128, 128], f16)
        ones64 = p_const.tile([1, H], f16)
        nc.vector.memset(ones64[:], 1.0)
        nc.sync.dma_start(id_sb[:], ident[:])
        for sb_t, ext in ((wv_sb, wv), (wk_sb, wk), (wq_sb, wq)):
            nc.sync.dma_start(
                sb_t[:].rearrange("p (c m) -> p c m", m=NW),
                ext[:].rearrange("(c p) m -> p c m", p=128),
            )
        for sb_t, ext in ((bv_sb, bv), (bk_sb, bk), (bq_sb, bq)):
            nc.sync.dma_start(sb_t[:], ext[:])

        a2i = [
            p_dram.tile([NCORES * NW, XO], f16, name=f"a2i{s}", tag=f"a2i{s}")
            for s in range(NBLK)
        ]
        a2o = [
            p_dram.tile([NCORES * NW, XO], f16, name=f"a2o{s}", tag=f"a2o{s}")
            for s in range(NBLK)
        ]
        # two dummy collectives issued at t~0: the first two CC-stream ops pay
        # ~10us trigger-warmup each, so burn them during the DMA preamble
        wrm_i = p_dram.tile([NCORES, XO], f16, name="wrm_i", tag="wrm_i")
        wrm_o = p_dram.tile([NCORES, XO], f16, name="wrm_o", tag="wrm_o")
        wrm_o2 = p_dram.tile([NCORES, XO], f16, name="wrm_o2", tag="wrm_o2")

        ps_sc = ctx.enter_context(tc.tile_pool(name="ps_sc", bufs=3, space="PSUM"))
        ps_pv = ctx.enter_context(tc.tile_pool(name="ps_pv", bufs=2, space="PSUM"))

        def load_xvk(b, xk, xv):
            for sb_t, ext in ((xv, vT[b]), (xk, kT[b])):
                for dc in range(NDC):
                    nc.sync.dma_start(
                        sb_t[:, dc * T : (dc + 1) * T],
                        ext[dc * 128 : (dc + 1) * 128, :],
                    )

        def load_xq(b, xq):
            # chunked by (tq-block, d-chunk) so Q-proj(j) starts after 1 MB
            for j in range(NTQB):
                for dc in range(NDC):
                    nc.sync.dma_start(
                        xq[:, dc * T + j * TQB : dc * T + (j + 1) * TQB],
                        qT[b][dc * 128 : (dc + 1) * 128, j * TQB : (j + 1) * TQB],
                    )

        def vk_proj(b, xk, xv):
            # V projection as V^T [nh, t] (N=512 matmuls), then PE
            # transposes into V_aug [tk, (i, head, 128)] with ones columns
            vt = p_va.tile([128, T], f16, name=f"vt{b}", tag="vt")
            for tb in range(NTQB):
                psv = ps_sc.tile([128, TQB], f32, name=f"psv{b}{tb}", tag="sc")
                for dc in range(NDC):
                    nc.tensor.matmul(
                        psv[:],
                        lhsT=wv_sb[:, dc * NW : (dc + 1) * NW],
                        rhs=xv[:, dc * T + tb * TQB : dc * T + (tb + 1) * TQB],
                        start=(dc == 0),
                        stop=(dc == NDC - 1),
                    )
                nc.vector.tensor_scalar(
                    vt[:, tb * TQB : (tb + 1) * TQB],
                    psv[:],
                    bv_sb[:, 0:1],
                    None,
                    ALU.add,
                )
            va = p_va.tile([128, NTKC * NLOC * VA], f16, name=f"va{b}", tag="va")
            nc.vector.memset(
                va[:].rearrange("p (i h x) -> p i h x", h=NLOC, x=VA)[
                    :, :, :, H : H + 1
                ],
                1.0,
            )
            for i in range(NTKC):
                pst = ps_sc.tile([128, 128], f16, name=f"pst{b}{i}", tag="sc")
                nc.tensor.transpose(
                    pst[:], vt[:, i * TKC : (i + 1) * TKC], id_sb[:]
                )
                dst = va[:, i * NLOC * VA : (i + 1) * NLOC * VA].rearrange(
                    "p (h x) -> p h x", x=VA
                )[:, :, 0:H]
                nc.vector.tensor_copy(
                    dst, pst[:].rearrange("p (h x) -> p h x", x=H)
                )

            # K^T projection (full, needed before any scores)
            kt = p_qk.tile([128, T], f16, name=f"kt{b}", tag="qk")
            for tb in range(NTQB):
                ps = ps_sc.tile([128, TQB], f32, name=f"psk{b}{tb}", tag="sc")
                for dc in range(NDC):
                    nc.tensor.matmul(
                        ps[:],
                        lhsT=wk_sb[:, dc * NW : (dc + 1) * NW],
                        rhs=xk[:, dc * T + tb * TQB : dc * T + (tb + 1) * TQB],
                        start=(dc == 0),
                        stop=(dc == NDC - 1),
                    )
                nc.vector.tensor_scalar(
                    kt[:, tb * TQB : (tb + 1) * TQB],
                    ps[:],
                    bk_sb[:, 0:1],
                    None,
                    ALU.add,
                )
            return kt, va

        def q_proj(b, j, xq, qt):
            ps = ps_sc.tile([128, TQB], f32, name=f"psq{b}{j}", tag="sc")
            for dc in range(NDC):
                nc.tensor.matmul(
                    ps[:],
                    lhsT=wq_sb[:, dc * NW : (dc + 1) * NW],
                    rhs=xq[:, dc * T + j * TQB : dc * T + (j + 1) * TQB],
                    start=(dc == 0),
                    stop=(dc == NDC - 1),
                )
            nc.vector.tensor_scalar(
                qt[:, j * TQB : (j + 1) * TQB],
                ps[:],
                bq_sb[:, 0:1],
                None,
                ALU.add,
            )

        # Bulk prefetch DMAs drain one-per-round so the latency-critical
        # staging transfers never sit behind megabytes of prefetch on the
        # DMA rings.
        prefq = []

        def pump():
            if prefq:
                dst, src = prefq.pop(0)
                nc.sync.dma_start(dst, src)

        def stage_dma(s, hd, an):
            nc.sync.dma_start(
                a2i[s][:].rearrange("(d h p) x -> h p d x", h=NLOC, p=H)[hd],
                an[:].rearrange("p (d x) -> p d x", x=XO),
            )

        def emit_cc(s):
            nc.gpsimd.collective_compute(
                "AllToAll",
                ALU.bypass,
                replica_groups=GROUPS,
                ins=[a2i[s].opt()],
                outs=[a2o[s].opt()],
            )

        def norm_fast(b, j, pv):
            """Normalize with no DRAM bounces: reciprocal as exp(-ln(x)) on
            the Scalar engine (Ln and Exp share one activation table set, so
            no table reload), partition-broadcast via a K=1 ones matmul on the
            PE, DVE multiply. All sub-us compute-engine hops (~2.5us total),
            so the chain neither rides the congested DMA rings nor parks
            long-latency waits on the DVE queue ahead of the exp CASTs."""
            s = b * NTQB + j
            for hd in range(NLOC):
                a_sb = p_a.tile([H + 1, TQB], f32, name=f"fa{b}{j}{hd}", tag="a")
                nc.vector.tensor_copy(a_sb[:], pv[hd][0 : H + 1, :])
                t32 = p_a.tile([1, TQB], f32, name=f"ft{b}{j}{hd}", tag="t32")
                nc.scalar.activation(t32[:], a_sb[H : H + 1, :], AF.Ln)
                rcf = p_a.tile([1, TQB], f16, name=f"fr{b}{j}{hd}", tag="rcf")
                nc.scalar.activation(rcf[:], t32[:], AF.Exp, 0.0, -1.0)
                rep = ps_sc.tile([H, TQB], f32, name=f"frp{b}{j}{hd}", tag="sc")
                nc.tensor.matmul(
                    rep[:], lhsT=ones64[0:1, :], rhs=rcf[0:1, :],
                    start=True, stop=True,
                )
                an = p_a.tile([H, TQB], f16, name=f"fan{b}{j}{hd}", tag="an")
                nc.vector.tensor_tensor(an[:], a_sb[0:H, :], rep[:], ALU.mult)
                stage_dma(s, hd, an)
            emit_cc(s)

        def attn_block(b, j, qt, kt, va):
            pv = [
                ps_pv.tile([VA, TQB], f32, name=f"pv{b}{j}{hd}", tag="pv")
                for hd in range(NLOC)
            ]
            pv_emitted = [0, 0]
            # dual-rounds: tiles r and r+1 (i = r), PVs emitted in
            # reversed tile order so the second pair needs no new wait
            # (covered by the ACT-queue wait of the first pair).
            for r0 in range(0, NTKC, 2):
                pts = []
                for r in (r0, r0 + 1):
                    pss = ps_sc.tile(
                        [128, SLOTS * TQB], f32, name=f"pss{b}{j}{r}", tag="sc"
                    )
                    for hd in range(NLOC):
                        nc.tensor.matmul(
                            pss[:, hd * TQB : (hd + 1) * TQB],
                            lhsT=kt[
                                hd * H : (hd + 1) * H, r * TKC : (r + 1) * TKC
                            ],
                            rhs=qt[
                                hd * H : (hd + 1) * H, j * TQB : (j + 1) * TQB
                            ],
                            start=True,
                            stop=True,
                        )
                    # exp evacuation: ACT reading PSUM throttles
                    # concurrent PE matmuls ~1.8x, DVE PSUM reads do
                    # not — but the DVE fp32->f16 CAST is 1x-slow, so
                    # alternate the two paths.
                    pt = p_pt.tile(
                        [128, SLOTS * TQB], f16, name=f"pt{b}{j}{r}", tag="pt"
                    )
                    if r % 2 == 0:
                        nc.scalar.activation(pt[:], pss[:], AF.Exp)
                    else:
                        s_sb = p_pt.tile(
                            [128, SLOTS * TQB],
                            f16,
                            name=f"ss{b}{j}{r}",
                            tag="ss",
                        )
                        nc.vector.tensor_copy(s_sb[:], pss[:])
                        nc.scalar.activation(pt[:], s_sb[:], AF.Exp)
                    pts.append((r, pt))
                for r, pt in reversed(pts):
                    for hd in range(NLOC):
                        col0 = r * NLOC * VA + hd * VA
                        nc.tensor.matmul(
                            pv[hd][:],
                            lhsT=va[:, col0 : col0 + VA],
                            rhs=pt[:, hd * TQB : (hd + 1) * TQB],
                            start=(pv_emitted[hd] == 0),
                            stop=(pv_emitted[hd] == NTKC - 1),
                        )
                        pv_emitted[hd] += 1
                pump()
            norm_fast(b, j, pv)

        def outproj(P):
            # blocks 2P and 2P+1 stacked: full 128-partition matmul
            at = p_at.tile([128, NHC * 2 * XO], f16, name=f"at{P}", tag="at")
            for s01 in range(2):
                s = 2 * P + s01
                nc.sync.dma_start(
                    at[:].rearrange("p (c s x) -> p c s x", s=2, x=XO)[
                        :, :, s01, :
                    ],
                    a2o[s][:].rearrange("(c p) x -> p c x", p=128),
                )
            for dh in range(2):
                pso = ps_sc.tile([128, 512], f32, name=f"pso{P}{dh}", tag="sc")
                for c in range(NHC):
                    nc.tensor.matmul(
                        pso[:],
                        lhsT=at[:, c * 2 * XO : (c + 1) * 2 * XO],
                        rhs=wp_sb[:, c * D + dh * 512 : c * D + dh * 512 + 512],
                        start=(c == 0),
                        stop=(c == NHC - 1),
                    )
                o_sb = p_o.tile([128, 512], f32, name=f"o{P}{dh}", tag="o")
                nc.vector.tensor_tensor(
                    o_sb[:],
                    pso[:],
                    bp_sb[:, dh * 512 : (dh + 1) * 512],
                    ALU.add,
                )
                nc.sync.dma_start(
                    out[2 * P : 2 * P + 2, :, dh * 512 : (dh + 1) * 512]
                    .rearrange("s p x -> (s p) x"),
                    o_sb[:],
                )

        # ===== schedule ======================================================
        xts = {}
        for b in range(B):
            xts[b] = (
                p_xt.tile([128, NDC * T], f16, name=f"xq{b}", tag="xq"),
                p_xt.tile([128, NDC * T], f16, name=f"xk{b}", tag="xk"),
                p_xt.tile([128, NDC * T], f16, name=f"xv{b}", tag="xv"),
            )

        for wo in (wrm_o, wrm_o2):
            nc.gpsimd.collective_compute(
                "AllToAll",
                ALU.bypass,
                replica_groups=GROUPS,
                ins=[wrm_i.opt()],
                outs=[wo.opt()],
            )
        load_xvk(0, xts[0][1], xts[0][2])
        load_xq(0, xts[0][0])
        kt0, va0 = vk_proj(0, xts[0][1], xts[0][2])

        # Deferred bulk loads (wp/bp, b=1 K/V activations): drained one chunk
        # per dual-round by pump(), so the latency-critical normalize/staging
        # transfers never sit behind megabytes of prefetch on the DMA rings.
        wp_sb = p_const.tile([128, NHC * D], f16, name="wp_sb")
        bp_sb = p_const.tile([128, D], f32, name="bp_sb")
        for sb_t, ext in ((xts[1][2], vT[1]), (xts[1][1], kT[1])):
            for dc in range(NDC):
                prefq.append(
                    (
                        sb_t[:, dc * T : (dc + 1) * T],
                        ext[dc * 128 : (dc + 1) * 128, :],
                    )
                )
        for c in range(NHC):
            prefq.append(
                (wp_sb[:, c * D : (c + 1) * D], wp[c * 128 : (c + 1) * 128, :])
            )
        prefq.append((bp_sb[:], bp[:]))

        qt0 = p_qk.tile([128, T], f16, name="qt0", tag="qk")
        for j in range(NTQB):
            q_proj(0, j, xts[0][0], qt0)
            attn_block(0, j, qt0, kt0, va0)
        outproj(0)
        # xq0's slot is fully read (q_proj(0,3) issued above); b=1 Q stream
        # lands during vk_proj(1)'s ~14us of PE work
        load_xq(1, xts[1][0])

        kt1, va1 = vk_proj(1, xts[1][1], xts[1][2])
        qt1 = p_qk.tile([128, T], f16, name="qt1", tag="qk")
        for j in range(NTQB):
            q_proj(1, j, xts[1][0], qt1)
            attn_block(1, j, qt1, kt1, va1)
            if j == 1:
                outproj(1)
        outproj(2)
        outproj(3)

    orig_to_json = nc.to_json_bytes
    nc.to_json_bytes = lambda: _legalize_waits(orig_to_json())
    return nc


def _get_nc():
    if "nc" not in _CACHE:
        _CACHE["nc"] = _build()
    return _CACHE["nc"]


def _make_in_maps(inputs):
    q = np.asarray(inputs["q"], dtype=np.float32)
    v = np.asarray(inputs["v"], dtype=np.float32)
    k = np.asarray(inputs["k"], dtype=np.float32)
    w_query = np.asarray(inputs["w_query"], dtype=np.float32)
    b_query = np.asarray(inputs["b_query"], dtype=np.float32)
    w_value = np.asarray(inputs["w_value"], dtype=np.float32)
    b_value = np.asarray(inputs["b_value"], dtype=np.float32)
    w_key = np.asarray(inputs["w_key"], dtype=np.float32)
    b_key = np.asarray(inputs["b_key"], dtype=np.float32)
    w_projection = np.asarray(inputs["w_projection"], dtype=np.float32)
    b_projection = np.asarray(inputs["b_projection"], dtype=np.float32)

    scale = np.float32(1.0 / np.sqrt(H))
    wp_s = np.ascontiguousarray(
        w_projection.transpose(0, 2, 1).reshape(N_HEADS * H, D)
    ).astype(F16)
    bp_s = np.ascontiguousarray(
        np.tile(b_projection.reshape(1, D), (128, 1))
    ).astype(np.float32)

    xT = {}
    for b in range(B):
        xT[b] = tuple(
            np.ascontiguousarray(x[b].T).astype(F16) for x in (q, k, v)
        )

    in_maps = []
    for c in range(NCORES):
        hs = c * NLOC
        wq_s = (w_query[:, hs : hs + NLOC, :].reshape(D, NW) * scale).astype(F16)
        wk_s = w_key[:, hs : hs + NLOC, :].reshape(D, NW).astype(F16)
        wv_s = w_value[:, hs : hs + NLOC, :].reshape(D, NW).astype(F16)
        bq_s = np.ascontiguousarray(
            (b_query[hs : hs + NLOC].reshape(NW) * scale).reshape(NW, 1)
        ).astype(np.float32)
        bk_s = np.ascontiguousarray(
            b_key[hs : hs + NLOC].reshape(NW, 1)
        ).astype(np.float32)
        bv_s = np.ascontiguousarray(
            b_value[hs : hs + NLOC].reshape(NW, 1)
        ).astype(np.float32)
        m = {
            "ident": np.eye(128, dtype=np.float32).astype(F16),
            "wq": np.ascontiguousarray(wq_s),
            "wk": np.ascontiguousarray(wk_s),
            "wv": np.ascontiguousarray(wv_s),
            "wp": wp_s,
            "bq": bq_s,
            "bk": bk_s,
            "bv": bv_s,
            "bp": bp_s,
        }
        for b in range(B):
            m[f"qT{b}"], m[f"kT{b}"], m[f"vT{b}"] = xT[b]
        in_maps.append(m)
    return in_maps


def _assemble(results):
    out = np.empty((B, T, D), np.float32)
    for c in range(NCORES):
        r = results[c]["out"]  # [NBLK, XO, D]
        for s in range(NBLK):
            b, j = divmod(s, NTQB)
            r0 = j * TQB + c * XO
            out[b, r0 : r0 + XO, :] = r[s]
    return out


def run(inputs, trace=False, **kwargs):
    from concourse.bass_utils import run_bass_kernel_spmd

    nc = _get_nc()
    in_maps = _make_in_maps(inputs)
    res = run_bass_kernel_spmd(
        nc, in_maps, list(range(NCORES)), trace=trace, **kwargs
    )
    return _assemble(res.results), res


def kernel(**inputs) -> np.ndarray:
    out, _ = run(inputs, trace=False)
    return out


# revision 29
# speedup vs baseline: 1.1578x; 1.1578x over previous
"""Bass/Tile TRN2 kernel: 16-head MHA (B=2, T=2048, D=1024, H=64) on 8 NeuronCores.

Sharding: 8-way tensor parallel over heads — core c handles heads {2c, 2c+1}
for BOTH batches. After attention each (batch, tq-block) "block" s (8 total)
is re-sharded so that core c owns COLUMN SLICE c (64 rows) of every block:
one small AllToAll per block ([8*128, 64] f16, 128 KB) fires as soon as that
block is normalized on all cores, fully overlapped with the remaining
attention compute. The output projection runs per block-PAIR (two 64-row
slices stacked into a full 128-partition matmul), also overlapped; only the
last block's AllToAll + projection remain on the tail.

Per-core device pipeline (all FLOPs on device):
  - Activation loads are chunked (xv/xk by d-chunk, xq by (tq-block, d-chunk))
    so the V projection starts on chunk 0 instead of after the full 12 MB
    preamble; Q projection is interleaved per tq-block into the attention
    loop. b=1 activations stream during b=0 attention.
  - QKV projections as f16 matmuls accumulating fp32 in PSUM; activations
    arrive pre-transposed ([D, T]) so the contraction dim d sits on SBUF
    partitions. 1/sqrt(H) is folded into Wq/bq on host.
  - Scores S^T[tk, tq] = K^T.T @ Q^T per head; the two heads are issued
    back-to-back as row-tiled (K=64, partitions 0-63 / 64-127) matmuls so they
    run concurrently on the PE array.
  - exp on ScalarE straight out of PSUM (3-bank [128,1536] tiles), bf16 out.
  - PV matmul with a ones-augmented V (65 stationary columns) so row 64 of the
    PV accumulator is the softmax denominator for free.
  - Normalize: DVE reciprocal of the denominator row straight out of PSUM,
    GPSIMD partition-broadcast across the 64 head-dim partitions, DVE multiply
    (no DRAM bounces).
  - Per-block AllToAll (see above), per-pair output projection, direct DMA of
    the [2*64, 512] result slabs to the output tensor.
Host does layout-only prep (transpose, f16 cast, weight slicing/replication)
and scatters the 8 cores' [8, 64, D] slabs into the full output.
"""

import os
import sys
from contextlib import ExitStack

import numpy as np

sys.path.insert(0, "/opt/trn_rl_repo")

import ml_dtypes  # noqa: E402

F16 = np.float16

B, T, D = 2, 2048, 1024
N_HEADS, H = 16, 64
NCORES = 8
GROUPS = [[0, 1, 2, 3, 4, 5, 6, 7]]
NLOC = 2            # heads per core
TQB = 512           # tq block size
NTQB = T // TQB     # 4
NBLK = B * NTQB     # 8 blocks total
TKC = 128           # tk chunk size
NTKC = T // TKC     # 16
DC = 128            # d chunk
NDC = D // DC       # 8
SLOTS = 2           # score-psum slots per exp tile ([128, 2*512] = 2 banks)
VA = 128            # V_aug stationary width: [V(64) | ones(1) | junk(63)]
NW = NLOC * H       # 128: projection output width per core
XO = 64             # output rows per core per block (column slice)
NHC = N_HEADS * H // 128  # 8 head-dim chunks in the output projection

INTERLEAVE_QPROJ = True

_CACHE = {}


def _legalize_waits(bir_bytes):
    """This toolchain's walrus accepts at most ONE semaphore wait per
    instruction ("Too many sync wait commands"). Tile's sem assignment emits
    several. Hoist all but one wait of each instruction onto same-engine NoOps
    inserted immediately before it (engines execute their stream in order, so
    waiting earlier on the same engine is equivalent)."""
    import json

    j = json.loads(bir_bytes)
    ctr = 0
    for fn in j["functions"]:
        for blk in fn["blocks"]:
            out = []
            for ins in blk["instructions"]:
                si = ins.get("sync_info")
                waits = (si or {}).get("on_wait") or []
                if len(waits) > 1:
                    for w in waits[:-1]:
                        ctr += 1
                        out.append(
                            {
                                "engine": ins["engine"],
                                "ins": [],
                                "outs": [],
                                "name": f"waitfix-{ctr}",
                                "opcode": "NoOp",
                                "sync_info": {"on_wait": [w], "on_update": []},
                            }
                        )
                    si["on_wait"] = [waits[-1]]
                out.append(ins)
            blk["instructions"] = out
    return json.dumps(j).encode()


def _build():
    import concourse.bass as bass
    import concourse.mybir as mybir
    import concourse.tile as tile

    f32 = mybir.dt.float32
    f16 = mybir.dt.float16
    AF = mybir.ActivationFunctionType
    ALU = mybir.AluOpType

    nc = bass.Bass(
        "TRN2", target_bir_lowering=False, debug=False, num_devices=NCORES
    )

    qT = [nc.dram_tensor(f"qT{b}", [D, T], f16, kind="ExternalInput") for b in range(B)]
    kT = [nc.dram_tensor(f"kT{b}", [D, T], f16, kind="ExternalInput") for b in range(B)]
    vT = [nc.dram_tensor(f"vT{b}", [D, T], f16, kind="ExternalInput") for b in range(B)]
    wq = nc.dram_tensor("wq", [D, NW], f16, kind="ExternalInput")
    wk = nc.dram_tensor("wk", [D, NW], f16, kind="ExternalInput")
    wv = nc.dram_tensor("wv", [D, NW], f16, kind="ExternalInput")
    wp = nc.dram_tensor("wp", [N_HEADS * H, D], f16, kind="ExternalInput")
    bq = nc.dram_tensor("bq", [128, 1], f32, kind="ExternalInput")
    bk = nc.dram_tensor("bk", [128, 1], f32, kind="ExternalInput")
    bv = nc.dram_tensor("bv", [128, 1], f32, kind="ExternalInput")
    bp = nc.dram_tensor("bp", [128, D], f32, kind="ExternalInput")
    ident = nc.dram_tensor("ident", [128, 128], f16, kind="ExternalInput")
    out = nc.dram_tensor("out", [NBLK, XO, D], f32, kind="ExternalOutput")

    with tile.TileContext(nc) as tc, ExitStack() as ctx:
        p_const = ctx.enter_context(tc.tile_pool(name="const", bufs=1))
        p_xt = ctx.enter_context(tc.tile_pool(name="xt", bufs=1))
        p_qk = ctx.enter_context(tc.tile_pool(name="qk", bufs=4))
        p_va = ctx.enter_context(tc.tile_pool(name="va", bufs=2))
        p_pt = ctx.enter_context(tc.tile_pool(name="pt", bufs=3))
        p_a = ctx.enter_context(tc.tile_pool(name="a", bufs=3))
        p_at = ctx.enter_context(tc.tile_pool(name="at", bufs=2))
        p_o = ctx.enter_context(tc.tile_pool(name="o", bufs=2))
        p_dram = ctx.enter_context(tc.tile_pool(name="dram", bufs=1, space="DRAM"))

        # ---- small constant loads (wp/bp deferred past the x0 loads) --------
        wq_sb = p_const.tile([128, NDC * NW], f16)
        wk_sb = p_const.tile([128, NDC * NW], f16)
        wv_sb = p_const.tile([128, NDC * NW], f16)
        bq_sb = p_const.tile([128, 1], f32)
        bk_sb = p_const.tile([128, 1], f32)
        bv_sb = p_const.tile([128, 1], f32)
        id_sb = p_const.tile([128, 128], f16)
        ones64 = p_const.tile([1, H], f16)
        nc.vector.memset(ones64[:], 1.0)
        nc.sync.dma_start(id_sb[:], ident[:])
        for sb_t, ext in ((wv_sb, wv), (wk_sb, wk), (wq_sb, wq)):
            nc.sync.dma_start(
                sb_t[:].rearrange("p (c m) -> p c m", m=NW),
                ext[:].rearrange("(c p) m -> p c m", p=128),
            )
        for sb_t, ext in ((bv_sb, bv), (bk_sb, bk), (bq_sb, bq)):
            nc.sync.dma_start(sb_t[:], ext[:])

        a2i = [
            p_dram.tile([NCORES * NW, XO], f16, name=f"a2i{s}", tag=f"a2i{s}")
            for s in range(NBLK)
        ]
        a2o = [
            p_dram.tile([NCORES * NW, XO], f16, name=f"a2o{s}", tag=f"a2o{s}")
            for s in range(NBLK)
        ]
        # two dummy collectives issued at t~0: the first two CC-stream ops pay
        # ~10us trigger-warmup each, so burn them during the DMA preamble
        wrm_i = p_dram.tile([NCORES, XO], f16, name="wrm_i", tag="wrm_i")
        wrm_o = p_dram.tile([NCORES, XO], f16, name="wrm_o", tag="wrm_o")
        wrm_o2 = p_dram.tile([NCORES, XO], f16, name="wrm_o2", tag="wrm_o2")

        ps_sc = ctx.enter_context(tc.tile_pool(name="ps_sc", bufs=3, space="PSUM"))
        ps_pv = ctx.enter_context(tc.tile_pool(name="ps_pv", bufs=2, space="PSUM"))

        def load_xvk(b, xk, xv):
            for sb_t, ext in ((xv, vT[b]), (xk, kT[b])):
                for dc in range(NDC):
                    nc.sync.dma_start(
                        sb_t[:, dc * T : (dc + 1) * T],
                        ext[dc * 128 : (dc + 1) * 128, :],
                    )

        def load_xq(b, xq):
            # chunked by (tq-block, d-chunk) so Q-proj(j) starts after 1 MB
            for j in range(NTQB):
                for dc in range(NDC):
                    nc.sync.dma_start(
                        xq[:, dc * T + j * TQB : dc * T + (j + 1) * TQB],
                        qT[b][dc * 128 : (dc + 1) * 128, j * TQB : (j + 1) * TQB],
                    )

        def vk_proj(b, xk, xv):
            # V projection as V^T [nh, t] (N=512 matmuls), then PE
            # transposes into V_aug [tk, (i, head, 128)] with ones columns
            vt = p_va.tile([128, T], f16, name=f"vt{b}", tag="vt")
            for tb in range(NTQB):
                psv = ps_sc.tile([128, TQB], f32, name=f"psv{b}{tb}", tag="sc")
                for dc in range(NDC):
                    nc.tensor.matmul(
                        psv[:],
                        lhsT=wv_sb[:, dc * NW : (dc + 1) * NW],
                        rhs=xv[:, dc * T + tb * TQB : dc * T + (tb + 1) * TQB],
                        start=(dc == 0),
                        stop=(dc == NDC - 1),
                    )
                nc.vector.tensor_scalar(
                    vt[:, tb * TQB : (tb + 1) * TQB],
                    psv[:],
                    bv_sb[:, 0:1],
                    None,
                    ALU.add,
                )
            va = p_va.tile([128, NTKC * NLOC * VA], f16, name=f"va{b}", tag="va")
            nc.vector.memset(
                va[:].rearrange("p (i h x) -> p i h x", h=NLOC, x=VA)[
                    :, :, :, H : H + 1
                ],
                1.0,
            )
            for i in range(NTKC):
                pst = ps_sc.tile([128, 128], f16, name=f"pst{b}{i}", tag="sc")
                nc.tensor.transpose(
                    pst[:], vt[:, i * TKC : (i + 1) * TKC], id_sb[:]
                )
                dst = va[:, i * NLOC * VA : (i + 1) * NLOC * VA].rearrange(
                    "p (h x) -> p h x", x=VA
                )[:, :, 0:H]
                nc.vector.tensor_copy(
                    dst, pst[:].rearrange("p (h x) -> p h x", x=H)
                )

            # K^T projection (full, needed before any scores)
            kt = p_qk.tile([128, T], f16, name=f"kt{b}", tag="qk")
            for tb in range(NTQB):
                ps = ps_sc.tile([128, TQB], f32, name=f"psk{b}{tb}", tag="sc")
                for dc in range(NDC):
                    nc.tensor.matmul(
                        ps[:],
                        lhsT=wk_sb[:, dc * NW : (dc + 1) * NW],
                        rhs=xk[:, dc * T + tb * TQB : dc * T + (tb + 1) * TQB],
                        start=(dc == 0),
                        stop=(dc == NDC - 1),
                    )
                nc.vector.tensor_scalar(
                    kt[:, tb * TQB : (tb + 1) * TQB],
                    ps[:],
                    bk_sb[:, 0:1],
                    None,
                    ALU.add,
                )
            return kt, va

        def q_proj(b, j, xq, qt, half=None, state={}):
            if half in (None, 0):
                state[(b, j)] = ps_sc.tile(
                    [128, TQB], f32, name=f"psq{b}{j}", tag="sc"
                )
            ps = state[(b, j)]
            dcs = range(NDC) if half is None else (
                range(NDC // 2) if half == 0 else range(NDC // 2, NDC)
            )
            for dc in dcs:
                nc.tensor.matmul(
                    ps[:],
                    lhsT=wq_sb[:, dc * NW : (dc + 1) * NW],
                    rhs=xq[:, dc * T + j * TQB : dc * T + (j + 1) * TQB],
                    start=(dc == 0),
                    stop=(dc == NDC - 1),
                )
            if half in (None, 1):
                nc.vector.tensor_scalar(
                    qt[:, j * TQB : (j + 1) * TQB],
                    ps[:],
                    bq_sb[:, 0:1],
                    None,
                    ALU.add,
                )
                state.pop((b, j))

        # Bulk prefetch DMAs drain one-per-round so the latency-critical
        # staging transfers never sit behind megabytes of prefetch on the
        # DMA rings.
        prefq = []

        def pump():
            if prefq:
                dst, src = prefq.pop(0)
                nc.sync.dma_start(dst, src)

        def stage_dma(s, hd, an):
            nc.sync.dma_start(
                a2i[s][:].rearrange("(d h p) x -> h p d x", h=NLOC, p=H)[hd],
                an[:].rearrange("p (d x) -> p d x", x=XO),
            )

        def emit_cc(s):
            nc.gpsimd.collective_compute(
                "AllToAll",
                ALU.bypass,
                replica_groups=GROUPS,
                ins=[a2i[s].opt()],
                outs=[a2o[s].opt()],
            )

        def norm_fast(b, j, pv):
            """Normalize with no DRAM bounces: reciprocal as exp(-ln(x)) on
            the Scalar engine (Ln and Exp share one activation table set, so
            no table reload), partition-broadcast via a K=1 ones matmul on the
            PE, DVE multiply. All sub-us compute-engine hops (~2.5us total),
            so the chain neither rides the congested DMA rings nor parks
            long-latency waits on the DVE queue ahead of the exp CASTs."""
            s = b * NTQB + j
            for hd in range(NLOC):
                a_sb = p_a.tile([H + 1, TQB], f32, name=f"fa{b}{j}{hd}", tag="a")
                nc.vector.tensor_copy(a_sb[:], pv[hd][0 : H + 1, :])
                t32 = p_a.tile([1, TQB], f32, name=f"ft{b}{j}{hd}", tag="t32")
                nc.scalar.activation(t32[:], a_sb[H : H + 1, :], AF.Ln)
                rcf = p_a.tile([1, TQB], f16, name=f"fr{b}{j}{hd}", tag="rcf")
                nc.scalar.activation(rcf[:], t32[:], AF.Exp, 0.0, -1.0)
                rep = ps_sc.tile([H, TQB], f32, name=f"frp{b}{j}{hd}", tag="sc")
                nc.tensor.matmul(
                    rep[:], lhsT=ones64[0:1, :], rhs=rcf[0:1, :],
                    start=True, stop=True,
                )
                an = p_a.tile([H, TQB], f16, name=f"fan{b}{j}{hd}", tag="an")
                nc.vector.tensor_tensor(an[:], a_sb[0:H, :], rep[:], ALU.mult)
                stage_dma(s, hd, an)
            emit_cc(s)
            # 7 pops x 4 b=0 blocks = 28 >= the 25 queued prefetch chunks:
            # everything must be emitted before vk_proj(1) reads xv1/xk1
            for _ in range(7):
                pump()

        def attn_block(b, j, qt, kt, va, next_qproj=None):
            pv = [
                ps_pv.tile([VA, TQB], f32, name=f"pv{b}{j}{hd}", tag="pv")
                for hd in range(NLOC)
            ]
            pv_emitted = [0, 0]
            # dual-rounds: tiles r and r+1 (i = r), PVs emitted in
            # reversed tile order so the second pair needs no new wait
            # (covered by the ACT-queue wait of the first pair).
            for r0 in range(0, NTKC, 2):
                pts = []
                for r in (r0, r0 + 1):
                    pss = ps_sc.tile(
                        [128, SLOTS * TQB], f32, name=f"pss{b}{j}{r}", tag="sc"
                    )
                    for hd in range(NLOC):
                        nc.tensor.matmul(
                            pss[:, hd * TQB : (hd + 1) * TQB],
                            lhsT=kt[
                                hd * H : (hd + 1) * H, r * TKC : (r + 1) * TKC
                            ],
                            rhs=qt[
                                hd * H : (hd + 1) * H, j * TQB : (j + 1) * TQB
                            ],
                            start=True,
                            stop=True,
                        )
                    # exp evacuation: ACT reading PSUM throttles
                    # concurrent PE matmuls ~1.8x, DVE PSUM reads do
                    # not — but the DVE fp32->f16 CAST is 1x-slow, so
                    # alternate the two paths.
                    pt = p_pt.tile(
                        [128, SLOTS * TQB], f16, name=f"pt{b}{j}{r}", tag="pt"
                    )
                    if r % 2 == 0:
                        nc.scalar.activation(pt[:], pss[:], AF.Exp)
                    else:
                        s_sb = p_pt.tile(
                            [128, SLOTS * TQB],
                            f16,
                            name=f"ss{b}{j}{r}",
                            tag="ss",
                        )
                        nc.vector.tensor_copy(s_sb[:], pss[:])
                        nc.scalar.activation(pt[:], s_sb[:], AF.Exp)
                    pts.append((r, pt))
                for r, pt in reversed(pts):
                    for hd in range(NLOC):
                        col0 = r * NLOC * VA + hd * VA
                        nc.tensor.matmul(
                            pv[hd][:],
                            lhsT=va[:, col0 : col0 + VA],
                            rhs=pt[:, hd * TQB : (hd + 1) * TQB],
                            start=(pv_emitted[hd] == 0),
                            stop=(pv_emitted[hd] == NTKC - 1),
                        )
                        pv_emitted[hd] += 1
                if r0 in (6, 8) and next_qproj is not None and INTERLEAVE_QPROJ:
                    # emit the NEXT block's Q projection mid-block (two 4-
                    # matmul halves): it runs in PE slack on a warm clock
                    # instead of serially at the (cold, ACT-idle) boundary.
                    # NOTE the psq tile holds an sc-ring slot from alloc to
                    # bias-read; only 2 pss allocations may intervene (ring=3)
                    next_qproj(0 if r0 == 6 else 1)
            norm_fast(b, j, pv)

        def outproj(P):
            # blocks 2P and 2P+1 stacked: full 128-partition matmul
            at = p_at.tile([128, NHC * 2 * XO], f16, name=f"at{P}", tag="at")
            for s01 in range(2):
                s = 2 * P + s01
                nc.sync.dma_start(
                    at[:].rearrange("p (c s x) -> p c s x", s=2, x=XO)[
                        :, :, s01, :
                    ],
                    a2o[s][:].rearrange("(c p) x -> p c x", p=128),
                )
            for dh in range(2):
                pso = ps_sc.tile([128, 512], f32, name=f"pso{P}{dh}", tag="sc")
                for c in range(NHC):
                    nc.tensor.matmul(
                        pso[:],
                        lhsT=at[:, c * 2 * XO : (c + 1) * 2 * XO],
                        rhs=wp_sb[:, c * D + dh * 512 : c * D + dh * 512 + 512],
                        start=(c == 0),
                        stop=(c == NHC - 1),
                    )
                o_sb = p_o.tile([128, 512], f32, name=f"o{P}{dh}", tag="o")
                nc.vector.tensor_tensor(
                    o_sb[:],
                    pso[:],
                    bp_sb[:, dh * 512 : (dh + 1) * 512],
                    ALU.add,
                )
                nc.sync.dma_start(
                    out[2 * P : 2 * P + 2, :, dh * 512 : (dh + 1) * 512]
                    .rearrange("s p x -> (s p) x"),
                    o_sb[:],
                )

        # ===== schedule ======================================================
        xts = {}
        for b in range(B):
            xts[b] = (
                p_xt.tile([128, NDC * T], f16, name=f"xq{b}", tag="xq"),
                p_xt.tile([128, NDC * T], f16, name=f"xk{b}", tag="xk"),
                p_xt.tile([128, NDC * T], f16, name=f"xv{b}", tag="xv"),
            )

        for wo in (wrm_o, wrm_o2):
            nc.gpsimd.collective_compute(
                "AllToAll",
                ALU.bypass,
                replica_groups=GROUPS,
                ins=[wrm_i.opt()],
                outs=[wo.opt()],
            )
        load_xvk(0, xts[0][1], xts[0][2])
        load_xq(0, xts[0][0])
        kt0, va0 = vk_proj(0, xts[0][1], xts[0][2])

        # Deferred bulk loads (wp/bp, b=1 K/V activations): drained one chunk
        # per dual-round by pump(), so the latency-critical normalize/staging
        # transfers never sit behind megabytes of prefetch on the DMA rings.
        wp_sb = p_const.tile([128, NHC * D], f16, name="wp_sb")
        bp_sb = p_const.tile([128, D], f32, name="bp_sb")
        for sb_t, ext in ((xts[1][2], vT[1]), (xts[1][1], kT[1])):
            for dc in range(NDC):
                prefq.append(
                    (
                        sb_t[:, dc * T : (dc + 1) * T],
                        ext[dc * 128 : (dc + 1) * 128, :],
                    )
                )
        for c in range(NHC):
            prefq.append(
                (wp_sb[:, c * D : (c + 1) * D], wp[c * 128 : (c + 1) * 128, :])
            )
        prefq.append((bp_sb[:], bp[:]))

        qt0 = p_qk.tile([128, T], f16, name="qt0", tag="qk")
        q_proj(0, 0, xts[0][0], qt0)
        for j in range(NTQB):
            nq = (
                (lambda h, jn=j + 1: q_proj(0, jn, xts[0][0], qt0, half=h))
                if j + 1 < NTQB
                else None
            )
            attn_block(0, j, qt0, kt0, va0, next_qproj=nq)
            if not INTERLEAVE_QPROJ and j + 1 < NTQB:
                q_proj(0, j + 1, xts[0][0], qt0)
        outproj(0)
        # xq0's slot is fully read (q_proj(0,3) issued above); b=1 Q stream
        # lands during vk_proj(1)'s ~14us of PE work
        load_xq(1, xts[1][0])

        kt1, va1 = vk_proj(1, xts[1][1], xts[1][2])
        qt1 = p_qk.tile([128, T], f16, name="qt1", tag="qk")
        q_proj(1, 0, xts[1][0], qt1)
        for j in range(NTQB):
            nq = (
                (lambda h, jn=j + 1: q_proj(1, jn, xts[1][0], qt1, half=h))
                if j + 1 < NTQB
                else None
            )
            attn_block(1, j, qt1, kt1, va1, next_qproj=nq)
            if not INTERLEAVE_QPROJ and j + 1 < NTQB:
                q_proj(1, j + 1, xts[1][0], qt1)
            if j == 1:
                outproj(1)
        outproj(2)
        outproj(3)

    orig_to_json = nc.to_json_bytes
    nc.to_json_bytes = lambda: _legalize_waits(orig_to_json())
    return nc


def _get_nc():
    if "nc" not in _CACHE:
        _CACHE["nc"] = _build()
    return _CACHE["nc"]


def _make_in_maps(inputs):
    q = np.asarray(inputs["q"], dtype=np.float32)
    v = np.asarray(inputs["v"], dtype=np.float32)
    k = np.asarray(inputs["k"], dtype=np.float32)
    w_query = np.asarray(inputs["w_query"], dtype=np.float32)
    b_query = np.asarray(inputs["b_query"], dtype=np.float32)
    w_value = np.asarray(inputs["w_value"], dtype=np.float32)
    b_value = np.asarray(inputs["b_value"], dtype=np.float32)
    w_key = np.asarray(inputs["w_key"], dtype=np.float32)
    b_key = np.asarray(inputs["b_key"], dtype=np.float32)
    w_projection = np.asarray(inputs["w_projection"], dtype=np.float32)
    b_projection = np.asarray(inputs["b_projection"], dtype=np.float32)

    scale = np.float32(1.0 / np.sqrt(H))
    wp_s = np.ascontiguousarray(
        w_projection.transpose(0, 2, 1).reshape(N_HEADS * H, D)
    ).astype(F16)
    bp_s = np.ascontiguousarray(
        np.tile(b_projection.reshape(1, D), (128, 1))
    ).astype(np.float32)

    xT = {}
    for b in range(B):
        xT[b] = tuple(
            np.ascontiguousarray(x[b].T).astype(F16) for x in (q, k, v)
        )

    in_maps = []
    for c in range(NCORES):
        hs = c * NLOC
        wq_s = (w_query[:, hs : hs + NLOC, :].reshape(D, NW) * scale).astype(F16)
        wk_s = w_key[:, hs : hs + NLOC, :].reshape(D, NW).astype(F16)
        wv_s = w_value[:, hs : hs + NLOC, :].reshape(D, NW).astype(F16)
        bq_s = np.ascontiguousarray(
            (b_query[hs : hs + NLOC].reshape(NW) * scale).reshape(NW, 1)
        ).astype(np.float32)
        bk_s = np.ascontiguousarray(
            b_key[hs : hs + NLOC].reshape(NW, 1)
        ).astype(np.float32)
        bv_s = np.ascontiguousarray(
            b_value[hs : hs + NLOC].reshape(NW, 1)
        ).astype(np.float32)
        m = {
            "ident": np.eye(128, dtype=np.float32).astype(F16),
            "wq": np.ascontiguousarray(wq_s),
            "wk": np.ascontiguousarray(wk_s),
            "wv": np.ascontiguousarray(wv_s),
            "wp": wp_s,
            "bq": bq_s,
            "bk": bk_s,
            "bv": bv_s,
            "bp": bp_s,
        }
        for b in range(B):
            m[f"qT{b}"], m[f"kT{b}"], m[f"vT{b}"] = xT[b]
        in_maps.append(m)
    return in_maps


def _assemble(results):
    out = np.empty((B, T, D), np.float32)
    for c in range(NCORES):
        r = results[c]["out"]  # [NBLK, XO, D]
        for s in range(NBLK):
            b, j = divmod(s, NTQB)
            r0 = j * TQB + c * XO
            out[b, r0 : r0 + XO, :] = r[s]
    return out


def run(inputs, trace=False, **kwargs):
    from concourse.bass_utils import run_bass_kernel_spmd

    nc = _get_nc()
    in_maps = _make_in_maps(inputs)
    res = run_bass_kernel_spmd(
        nc, in_maps, list(range(NCORES)), trace=trace, **kwargs
    )
    return _assemble(res.results), res


def kernel(**inputs) -> np.ndarray:
    out, _ = run(inputs, trace=False)
    return out


# revision 35
# speedup vs baseline: 1.1822x; 1.0211x over previous
"""Bass/Tile TRN2 kernel: 16-head MHA (B=2, T=2048, D=1024, H=64) on 8 NeuronCores.

Sharding: 8-way tensor parallel over heads — core c handles heads {2c, 2c+1}
for BOTH batches. After attention each (batch, tq-block) "block" s (8 total)
is re-sharded so that core c owns COLUMN SLICE c (64 rows) of every block:
one small AllToAll per block ([8*128, 64] f16, 128 KB) fires as soon as that
block is normalized on all cores, fully overlapped with the remaining
attention compute. The output projection runs per block-PAIR (two 64-row
slices stacked into a full 128-partition matmul), also overlapped; only the
last block's AllToAll + projection remain on the tail.

Per-core device pipeline (all FLOPs on device):
  - Activation loads are chunked (xv/xk by d-chunk, xq by (tq-block, d-chunk))
    so the V projection starts on chunk 0 instead of after the full 12 MB
    preamble; Q projection is interleaved per tq-block into the attention
    loop. b=1 activations stream during b=0 attention.
  - QKV projections as f16 matmuls accumulating fp32 in PSUM; activations
    arrive pre-transposed ([D, T]) so the contraction dim d sits on SBUF
    partitions. 1/sqrt(H) is folded into Wq/bq on host.
  - Scores S^T[tk, tq] = K^T.T @ Q^T per head; the two heads are issued
    back-to-back as row-tiled (K=64, partitions 0-63 / 64-127) matmuls so they
    run concurrently on the PE array.
  - exp on ScalarE straight out of PSUM (3-bank [128,1536] tiles), bf16 out.
  - PV matmul with a ones-augmented V (65 stationary columns) so row 64 of the
    PV accumulator is the softmax denominator for free.
  - Normalize: DVE reciprocal of the denominator row straight out of PSUM,
    GPSIMD partition-broadcast across the 64 head-dim partitions, DVE multiply
    (no DRAM bounces).
  - Per-block AllToAll (see above), per-pair output projection, direct DMA of
    the [2*64, 512] result slabs to the output tensor.
Host does layout-only prep (transpose, f16 cast, weight slicing/replication)
and scatters the 8 cores' [8, 64, D] slabs into the full output.
"""

import os
import sys
from contextlib import ExitStack

import numpy as np

sys.path.insert(0, "/opt/trn_rl_repo")

import ml_dtypes  # noqa: E402

F16 = np.float16

B, T, D = 2, 2048, 1024
N_HEADS, H = 16, 64
NCORES = 8
GROUPS = [[0, 1, 2, 3, 4, 5, 6, 7]]
NLOC = 2            # heads per core
TQB = 512           # tq block size
NTQB = T // TQB     # 4
NBLK = B * NTQB     # 8 blocks total
TKC = 128           # tk chunk size
NTKC = T // TKC     # 16
DC = 128            # d chunk
NDC = D // DC       # 8
SLOTS = 2           # score-psum slots per exp tile ([128, 2*512] = 2 banks)
VA = 128            # V_aug stationary width: [V(64) | ones(1) | junk(63)]
NW = NLOC * H       # 128: projection output width per core
XO = 64             # output rows per core per block (column slice)
NHC = N_HEADS * H // 128  # 8 head-dim chunks in the output projection

INTERLEAVE_QPROJ = True

_CACHE = {}


def _legalize_waits(bir_bytes):
    """This toolchain's walrus accepts at most ONE semaphore wait per
    instruction ("Too many sync wait commands"). Tile's sem assignment emits
    several. Hoist all but one wait of each instruction onto same-engine NoOps
    inserted immediately before it (engines execute their stream in order, so
    waiting earlier on the same engine is equivalent)."""
    import json

    j = json.loads(bir_bytes)
    ctr = 0
    for fn in j["functions"]:
        for blk in fn["blocks"]:
            out = []
            for ins in blk["instructions"]:
                si = ins.get("sync_info")
                waits = (si or {}).get("on_wait") or []
                if len(waits) > 1:
                    for w in waits[:-1]:
                        ctr += 1
                        out.append(
                            {
                                "engine": ins["engine"],
                                "ins": [],
                                "outs": [],
                                "name": f"waitfix-{ctr}",
                                "opcode": "NoOp",
                                "sync_info": {"on_wait": [w], "on_update": []},
                            }
                        )
                    si["on_wait"] = [waits[-1]]
                out.append(ins)
            blk["instructions"] = out
    return json.dumps(j).encode()


def _build():
    import concourse.bass as bass
    import concourse.mybir as mybir
    import concourse.tile as tile

    f32 = mybir.dt.float32
    f16 = mybir.dt.float16
    AF = mybir.ActivationFunctionType
    ALU = mybir.AluOpType

    nc = bass.Bass(
        "TRN2", target_bir_lowering=False, debug=False, num_devices=NCORES
    )

    qT = [nc.dram_tensor(f"qT{b}", [D, T], f16, kind="ExternalInput") for b in range(B)]
    kT = [nc.dram_tensor(f"kT{b}", [D, T], f16, kind="ExternalInput") for b in range(B)]
    vT = [nc.dram_tensor(f"vT{b}", [D, T], f16, kind="ExternalInput") for b in range(B)]
    wq = nc.dram_tensor("wq", [D, NW], f16, kind="ExternalInput")
    wk = nc.dram_tensor("wk", [D, NW], f16, kind="ExternalInput")
    wv = nc.dram_tensor("wv", [D, NW], f16, kind="ExternalInput")
    wp = nc.dram_tensor("wp", [N_HEADS * H, D], f16, kind="ExternalInput")
    bq = nc.dram_tensor("bq", [128, 1], f32, kind="ExternalInput")
    bk = nc.dram_tensor("bk", [128, 1], f32, kind="ExternalInput")
    bv = nc.dram_tensor("bv", [128, 1], f32, kind="ExternalInput")
    bp = nc.dram_tensor("bp", [128, D], f32, kind="ExternalInput")
    ident = nc.dram_tensor("ident", [128, 128], f16, kind="ExternalInput")
    out = nc.dram_tensor("out", [NBLK, XO, D], f32, kind="ExternalOutput")

    with tile.TileContext(nc) as tc, ExitStack() as ctx:
        p_const = ctx.enter_context(tc.tile_pool(name="const", bufs=1))
        p_xt = ctx.enter_context(tc.tile_pool(name="xt", bufs=1))
        p_qk = ctx.enter_context(tc.tile_pool(name="qk", bufs=4))
        p_va = ctx.enter_context(tc.tile_pool(name="va", bufs=2))
        p_pt = ctx.enter_context(tc.tile_pool(name="pt", bufs=3))
        p_a = ctx.enter_context(tc.tile_pool(name="a", bufs=3))
        p_at = ctx.enter_context(tc.tile_pool(name="at", bufs=2))
        p_o = ctx.enter_context(tc.tile_pool(name="o", bufs=2))
        p_dram = ctx.enter_context(tc.tile_pool(name="dram", bufs=1, space="DRAM"))

        # ---- small constant loads (wp/bp deferred past the x0 loads) --------
        wq_sb = p_const.tile([128, NDC * NW], f16)
        wk_sb = p_const.tile([128, NDC * NW], f16)
        wv_sb = p_const.tile([128, NDC * NW], f16)
        bq_sb = p_const.tile([128, 1], f32)
        bk_sb = p_const.tile([128, 1], f32)
        bv_sb = p_const.tile([128, 1], f32)
        id_sb = p_const.tile([128, 128], f16)
        ones64 = p_const.tile([1, H], f16)
        nc.vector.memset(ones64[:], 1.0)
        nc.sync.dma_start(id_sb[:], ident[:])
        for sb_t, ext in ((wv_sb, wv), (wk_sb, wk), (wq_sb, wq)):
            nc.sync.dma_start(
                sb_t[:].rearrange("p (c m) -> p c m", m=NW),
                ext[:].rearrange("(c p) m -> p c m", p=128),
            )
        for sb_t, ext in ((bv_sb, bv), (bk_sb, bk), (bq_sb, bq)):
            nc.sync.dma_start(sb_t[:], ext[:])

        a2i = [
            p_dram.tile([NCORES * NW, XO], f16, name=f"a2i{s}", tag=f"a2i{s}")
            for s in range(NBLK)
        ]
        a2o = [
            p_dram.tile([NCORES * NW, XO], f16, name=f"a2o{s}", tag=f"a2o{s}")
            for s in range(NBLK)
        ]
        # two dummy collectives issued at t~0: the first two CC-stream ops pay
        # ~10us trigger-warmup each, so burn them during the DMA preamble
        wrm_i = p_dram.tile([NCORES, XO], f16, name="wrm_i", tag="wrm_i")
        wrm_o = p_dram.tile([NCORES, XO], f16, name="wrm_o", tag="wrm_o")
        wrm_o2 = p_dram.tile([NCORES, XO], f16, name="wrm_o2", tag="wrm_o2")

        ps_sc = ctx.enter_context(tc.tile_pool(name="ps_sc", bufs=3, space="PSUM"))
        ps_pv = ctx.enter_context(tc.tile_pool(name="ps_pv", bufs=2, space="PSUM"))

        def load_xvk(b, xk, xv):
            for sb_t, ext in ((xv, vT[b]), (xk, kT[b])):
                for dc in range(NDC):
                    nc.sync.dma_start(
                        sb_t[:, dc * T : (dc + 1) * T],
                        ext[dc * 128 : (dc + 1) * 128, :],
                    )

        def load_xq(b, xq):
            # chunked by (tq-block, d-chunk) so Q-proj(j) starts after 1 MB
            for j in range(NTQB):
                for dc in range(NDC):
                    nc.sync.dma_start(
                        xq[:, dc * T + j * TQB : dc * T + (j + 1) * TQB],
                        qT[b][dc * 128 : (dc + 1) * 128, j * TQB : (j + 1) * TQB],
                    )

        def mk_vk_pieces(b, xk, xv, st):
            """vk_proj split into 8 self-contained emitter pieces (each opens
            and closes its own PSUM group) so b=1's V/K projections can be
            interleaved into b=0's ACT-bound attention blocks."""

            def vtb(tb):
                if tb == 0:
                    st["vt"] = p_va.tile([128, T], f16, name=f"vt{b}", tag="vt")
                vt = st["vt"]
                psv = ps_sc.tile([128, TQB], f32, name=f"psv{b}{tb}", tag="sc")
                for dc in range(NDC):
                    nc.tensor.matmul(
                        psv[:],
                        lhsT=wv_sb[:, dc * NW : (dc + 1) * NW],
                        rhs=xv[:, dc * T + tb * TQB : dc * T + (tb + 1) * TQB],
                        start=(dc == 0),
                        stop=(dc == NDC - 1),
                    )
                nc.vector.tensor_scalar(
                    vt[:, tb * TQB : (tb + 1) * TQB],
                    psv[:],
                    bv_sb[:, 0:1],
                    None,
                    ALU.add,
                )

            def transp(i0):
                if i0 == 0:
                    st["va"] = p_va.tile(
                        [128, NTKC * NLOC * VA], f16, name=f"va{b}", tag="va"
                    )
                    nc.vector.memset(
                        st["va"][:].rearrange(
                            "p (i h x) -> p i h x", h=NLOC, x=VA
                        )[:, :, :, H : H + 1],
                        1.0,
                    )
                va = st["va"]
                for i in range(i0, i0 + NTKC // 2):
                    pst = ps_sc.tile([128, 128], f16, name=f"pst{b}{i}", tag="sc")
                    nc.tensor.transpose(
                        pst[:], st["vt"][:, i * TKC : (i + 1) * TKC], id_sb[:]
                    )
                    dst = va[:, i * NLOC * VA : (i + 1) * NLOC * VA].rearrange(
                        "p (h x) -> p h x", x=VA
                    )[:, :, 0:H]
                    nc.vector.tensor_copy(
                        dst, pst[:].rearrange("p (h x) -> p h x", x=H)
                    )

            def ktb(tb):
                if tb == 0:
                    st["kt"] = p_qk.tile([128, T], f16, name=f"kt{b}", tag="qk")
                kt = st["kt"]
                ps = ps_sc.tile([128, TQB], f32, name=f"psk{b}{tb}", tag="sc")
                for dc in range(NDC):
                    nc.tensor.matmul(
                        ps[:],
                        lhsT=wk_sb[:, dc * NW : (dc + 1) * NW],
                        rhs=xk[:, dc * T + tb * TQB : dc * T + (tb + 1) * TQB],
                        start=(dc == 0),
                        stop=(dc == NDC - 1),
                    )
                nc.vector.tensor_scalar(
                    kt[:, tb * TQB : (tb + 1) * TQB],
                    ps[:],
                    bk_sb[:, 0:1],
                    None,
                    ALU.add,
                )

            return [
                lambda: vtb(0),
                lambda: vtb(1),
                lambda: vtb(2),
                lambda: vtb(3),
                lambda: (transp(0), ktb(0)),
                lambda: (transp(8), ktb(1)),
                lambda: ktb(2),
                lambda: ktb(3),
            ]

        def vk_proj(b, xk, xv):
            st = {}
            for piece in mk_vk_pieces(b, xk, xv, st):
                piece()
            return st["kt"], st["va"]

        def q_proj(b, j, xq, qt, half=None, state={}):
            if half in (None, 0):
                state[(b, j)] = ps_sc.tile(
                    [128, TQB], f32, name=f"psq{b}{j}", tag="sc"
                )
            ps = state[(b, j)]
            dcs = range(NDC) if half is None else (
                range(NDC // 2) if half == 0 else range(NDC // 2, NDC)
            )
            for dc in dcs:
                nc.tensor.matmul(
                    ps[:],
                    lhsT=wq_sb[:, dc * NW : (dc + 1) * NW],
                    rhs=xq[:, dc * T + j * TQB : dc * T + (j + 1) * TQB],
                    start=(dc == 0),
                    stop=(dc == NDC - 1),
                )
            if half in (None, 1):
                nc.vector.tensor_scalar(
                    qt[:, j * TQB : (j + 1) * TQB],
                    ps[:],
                    bq_sb[:, 0:1],
                    None,
                    ALU.add,
                )
                state.pop((b, j))

        # Bulk prefetch DMAs drain one-per-round so the latency-critical
        # staging transfers never sit behind megabytes of prefetch on the
        # DMA rings.
        prefq = []
        peq = []

        def pump():
            if prefq:
                dst, src = prefq.pop(0)
                nc.sync.dma_start(dst, src)

        def stage_dma(s, hd, an):
            nc.sync.dma_start(
                a2i[s][:].rearrange("(d h p) x -> h p d x", h=NLOC, p=H)[hd],
                an[:].rearrange("p (d x) -> p d x", x=XO),
            )

        def emit_cc(s):
            nc.gpsimd.collective_compute(
                "AllToAll",
                ALU.bypass,
                replica_groups=GROUPS,
                ins=[a2i[s].opt()],
                outs=[a2o[s].opt()],
            )

        def norm_fast(b, j, pv):
            """Normalize with no DRAM bounces: reciprocal as exp(-ln(x)) on
            the Scalar engine (Ln and Exp share one activation table set, so
            no table reload), partition-broadcast via a K=1 ones matmul on the
            PE, DVE multiply. All sub-us compute-engine hops (~2.5us total),
            so the chain neither rides the congested DMA rings nor parks
            long-latency waits on the DVE queue ahead of the exp CASTs."""
            s = b * NTQB + j
            for hd in range(NLOC):
                a_sb = p_a.tile([H + 1, TQB], f32, name=f"fa{b}{j}{hd}", tag="a")
                nc.vector.tensor_copy(a_sb[:], pv[hd][0 : H + 1, :])
                t32 = p_a.tile([1, TQB], f32, name=f"ft{b}{j}{hd}", tag="t32")
                nc.scalar.activation(t32[:], a_sb[H : H + 1, :], AF.Ln)
                rcf = p_a.tile([1, TQB], f16, name=f"fr{b}{j}{hd}", tag="rcf")
                nc.scalar.activation(rcf[:], t32[:], AF.Exp, 0.0, -1.0)
                rep = ps_sc.tile([H, TQB], f32, name=f"frp{b}{j}{hd}", tag="sc")
                nc.tensor.matmul(
                    rep[:], lhsT=ones64[0:1, :], rhs=rcf[0:1, :],
                    start=True, stop=True,
                )
                an = p_a.tile([H, TQB], f16, name=f"fan{b}{j}{hd}", tag="an")
                nc.vector.tensor_tensor(an[:], a_sb[0:H, :], rep[:], ALU.mult)
                stage_dma(s, hd, an)
            emit_cc(s)
            # 7 pops x 4 b=0 blocks = 28 >= the 25 queued prefetch chunks:
            # everything must be emitted before vk_proj(1) reads xv1/xk1
            for _ in range(7):
                pump()

        def attn_block(b, j, qt, kt, va, next_qproj=None, allow_pe_pieces=False):
            pv = [
                ps_pv.tile([VA, TQB], f32, name=f"pv{b}{j}{hd}", tag="pv")
                for hd in range(NLOC)
            ]
            pv_emitted = [0, 0]
            # dual-rounds: tiles r and r+1 (i = r), PVs emitted in
            # reversed tile order so the second pair needs no new wait
            # (covered by the ACT-queue wait of the first pair).
            for r0 in range(0, NTKC, 2):
                pts = []
                for r in (r0, r0 + 1):
                    pss = ps_sc.tile(
                        [128, SLOTS * TQB], f32, name=f"pss{b}{j}{r}", tag="sc"
                    )
                    for hd in range(NLOC):
                        nc.tensor.matmul(
                            pss[:, hd * TQB : (hd + 1) * TQB],
                            lhsT=kt[
                                hd * H : (hd + 1) * H, r * TKC : (r + 1) * TKC
                            ],
                            rhs=qt[
                                hd * H : (hd + 1) * H, j * TQB : (j + 1) * TQB
                            ],
                            start=True,
                            stop=True,
                        )
                    # exp evacuation: ACT reading PSUM throttles
                    # concurrent PE matmuls ~1.8x, DVE PSUM reads do
                    # not — but the DVE fp32->f16 CAST is 1x-slow, so
                    # alternate the two paths.
                    pt = p_pt.tile(
                        [128, SLOTS * TQB], f16, name=f"pt{b}{j}{r}", tag="pt"
                    )
                    if r % 2 == 0:
                        nc.scalar.activation(pt[:], pss[:], AF.Exp)
                    else:
                        s_sb = p_pt.tile(
                            [128, SLOTS * TQB],
                            f16,
                            name=f"ss{b}{j}{r}",
                            tag="ss",
                        )
                        nc.vector.tensor_copy(s_sb[:], pss[:])
                        nc.scalar.activation(pt[:], s_sb[:], AF.Exp)
                    pts.append((r, pt))
                for r, pt in reversed(pts):
                    for hd in range(NLOC):
                        col0 = r * NLOC * VA + hd * VA
                        nc.tensor.matmul(
                            pv[hd][:],
                            lhsT=va[:, col0 : col0 + VA],
                            rhs=pt[:, hd * TQB : (hd + 1) * TQB],
                            start=(pv_emitted[hd] == 0),
                            stop=(pv_emitted[hd] == NTKC - 1),
                        )
                        pv_emitted[hd] += 1
                if r0 in (6, 8) and next_qproj is not None and INTERLEAVE_QPROJ:
                    # emit the NEXT block's Q projection mid-block (two 4-
                    # matmul halves): it runs in PE slack on a warm clock
                    # instead of serially at the (cold, ACT-idle) boundary.
                    # NOTE the psq tile holds an sc-ring slot from alloc to
                    # bias-read; only 2 pss allocations may intervene (ring=3)
                    next_qproj(0 if r0 == 6 else 1)
                if r0 in (4, 10, 12, 14) and allow_pe_pieces and peq:
                    # one self-contained b=1 V/K-projection piece per round:
                    # runs in PE slack instead of serially at the b0->b1
                    # transition
                    peq.pop(0)()
            norm_fast(b, j, pv)

        def outproj(P):
            # blocks 2P and 2P+1 stacked: full 128-partition matmul
            at = p_at.tile([128, NHC * 2 * XO], f16, name=f"at{P}", tag="at")
            for s01 in range(2):
                s = 2 * P + s01
                nc.sync.dma_start(
                    at[:].rearrange("p (c s x) -> p c s x", s=2, x=XO)[
                        :, :, s01, :
                    ],
                    a2o[s][:].rearrange("(c p) x -> p c x", p=128),
                )
            for dh in range(2):
                pso = ps_sc.tile([128, 512], f32, name=f"pso{P}{dh}", tag="sc")
                for c in range(NHC):
                    nc.tensor.matmul(
                        pso[:],
                        lhsT=at[:, c * 2 * XO : (c + 1) * 2 * XO],
                        rhs=wp_sb[:, c * D + dh * 512 : c * D + dh * 512 + 512],
                        start=(c == 0),
                        stop=(c == NHC - 1),
                    )
                o_sb = p_o.tile([128, 512], f32, name=f"o{P}{dh}", tag="o")
                nc.vector.tensor_tensor(
                    o_sb[:],
                    pso[:],
                    bp_sb[:, dh * 512 : (dh + 1) * 512],
                    ALU.add,
                )
                nc.sync.dma_start(
                    out[2 * P : 2 * P + 2, :, dh * 512 : (dh + 1) * 512]
                    .rearrange("s p x -> (s p) x"),
                    o_sb[:],
                )

        # ===== schedule ======================================================
        xts = {}
        for b in range(B):
            xts[b] = (
                p_xt.tile([128, NDC * T], f16, name=f"xq{b}", tag="xq"),
                p_xt.tile([128, NDC * T], f16, name=f"xk{b}", tag="xk"),
                p_xt.tile([128, NDC * T], f16, name=f"xv{b}", tag="xv"),
            )

        for wo in (wrm_o, wrm_o2):
            nc.gpsimd.collective_compute(
                "AllToAll",
                ALU.bypass,
                replica_groups=GROUPS,
                ins=[wrm_i.opt()],
                outs=[wo.opt()],
            )
        load_xvk(0, xts[0][1], xts[0][2])
        load_xq(0, xts[0][0])
        kt0, va0 = vk_proj(0, xts[0][1], xts[0][2])

        # Deferred bulk loads (wp/bp, b=1 K/V activations): drained one chunk
        # per dual-round by pump(), so the latency-critical normalize/staging
        # transfers never sit behind megabytes of prefetch on the DMA rings.
        wp_sb = p_const.tile([128, NHC * D], f16, name="wp_sb")
        bp_sb = p_const.tile([128, D], f32, name="bp_sb")
        for sb_t, ext in ((xts[1][2], vT[1]), (xts[1][1], kT[1])):
            for dc in range(NDC):
                prefq.append(
                    (
                        sb_t[:, dc * T : (dc + 1) * T],
                        ext[dc * 128 : (dc + 1) * 128, :],
                    )
                )
        for c in range(NHC):
            prefq.append(
                (wp_sb[:, c * D : (c + 1) * D], wp[c * 128 : (c + 1) * 128, :])
            )
        prefq.append((bp_sb[:], bp[:]))

        # b=1 V/K projection pieces, interleaved into b0 blocks (0,2)/(0,3)
        st1 = {}
        peq.extend(mk_vk_pieces(1, xts[1][1], xts[1][2], st1))

        qt0 = p_qk.tile([128, T], f16, name="qt0", tag="qk")
        q_proj(0, 0, xts[0][0], qt0)
        for j in range(NTQB):
            nq = (
                (lambda h, jn=j + 1: q_proj(0, jn, xts[0][0], qt0, half=h))
                if j + 1 < NTQB
                else None
            )
            attn_block(0, j, qt0, kt0, va0, next_qproj=nq,
                       allow_pe_pieces=(j >= 2))
            if not INTERLEAVE_QPROJ and j + 1 < NTQB:
                q_proj(0, j + 1, xts[0][0], qt0)
        while peq:
            peq.pop(0)()
        outproj(0)
        # xq0's slot is fully read (q_proj(0,3) issued above)
        load_xq(1, xts[1][0])

        kt1, va1 = st1["kt"], st1["va"]
        qt1 = p_qk.tile([128, T], f16, name="qt1", tag="qk")
        q_proj(1, 0, xts[1][0], qt1)
        for j in range(NTQB):
            nq = (
                (lambda h, jn=j + 1: q_proj(1, jn, xts[1][0], qt1, half=h))
                if j + 1 < NTQB
                else None
            )
            attn_block(1, j, qt1, kt1, va1, next_qproj=nq)
            if not INTERLEAVE_QPROJ and j + 1 < NTQB:
                q_proj(1, j + 1, xts[1][0], qt1)
            if j == 1:
                outproj(1)
        outproj(2)
        outproj(3)

    orig_to_json = nc.to_json_bytes
    nc.to_json_bytes = lambda: _legalize_waits(orig_to_json())
    return nc


def _get_nc():
    if "nc" not in _CACHE:
        _CACHE["nc"] = _build()
    return _CACHE["nc"]


def _make_in_maps(inputs):
    q = np.asarray(inputs["q"], dtype=np.float32)
    v = np.asarray(inputs["v"], dtype=np.float32)
    k = np.asarray(inputs["k"], dtype=np.float32)
    w_query = np.asarray(inputs["w_query"], dtype=np.float32)
    b_query = np.asarray(inputs["b_query"], dtype=np.float32)
    w_value = np.asarray(inputs["w_value"], dtype=np.float32)
    b_value = np.asarray(inputs["b_value"], dtype=np.float32)
    w_key = np.asarray(inputs["w_key"], dtype=np.float32)
    b_key = np.asarray(inputs["b_key"], dtype=np.float32)
    w_projection = np.asarray(inputs["w_projection"], dtype=np.float32)
    b_projection = np.asarray(inputs["b_projection"], dtype=np.float32)

    scale = np.float32(1.0 / np.sqrt(H))
    wp_s = np.ascontiguousarray(
        w_projection.transpose(0, 2, 1).reshape(N_HEADS * H, D)
    ).astype(F16)
    bp_s = np.ascontiguousarray(
        np.tile(b_projection.reshape(1, D), (128, 1))
    ).astype(np.float32)

    xT = {}
    for b in range(B):
        xT[b] = tuple(
            np.ascontiguousarray(x[b].T).astype(F16) for x in (q, k, v)
        )

    in_maps = []
    for c in range(NCORES):
        hs = c * NLOC
        wq_s = (w_query[:, hs : hs + NLOC, :].reshape(D, NW) * scale).astype(F16)
        wk_s = w_key[:, hs : hs + NLOC, :].reshape(D, NW).astype(F16)
        wv_s = w_value[:, hs : hs + NLOC, :].reshape(D, NW).astype(F16)
        bq_s = np.ascontiguousarray(
            (b_query[hs : hs + NLOC].reshape(NW) * scale).reshape(NW, 1)
        ).astype(np.float32)
        bk_s = np.ascontiguousarray(
            b_key[hs : hs + NLOC].reshape(NW, 1)
        ).astype(np.float32)
        bv_s = np.ascontiguousarray(
            b_value[hs : hs + NLOC].reshape(NW, 1)
        ).astype(np.float32)
        m = {
            "ident": np.eye(128, dtype=np.float32).astype(F16),
            "wq": np.ascontiguousarray(wq_s),
            "wk": np.ascontiguousarray(wk_s),
            "wv": np.ascontiguousarray(wv_s),
            "wp": wp_s,
            "bq": bq_s,
            "bk": bk_s,
            "bv": bv_s,
            "bp": bp_s,
        }
        for b in range(B):
            m[f"qT{b}"], m[f"kT{b}"], m[f"vT{b}"] = xT[b]
        in_maps.append(m)
    return in_maps


def _assemble(results):
    out = np.empty((B, T, D), np.float32)
    for c in range(NCORES):
        r = results[c]["out"]  # [NBLK, XO, D]
        for s in range(NBLK):
            b, j = divmod(s, NTQB)
            r0 = j * TQB + c * XO
            out[b, r0 : r0 + XO, :] = r[s]
    return out


def run(inputs, trace=False, **kwargs):
    from concourse.bass_utils import run_bass_kernel_spmd

    nc = _get_nc()
    in_maps = _make_in_maps(inputs)
    res = run_bass_kernel_spmd(
        nc, in_maps, list(range(NCORES)), trace=trace, **kwargs
    )
    return _assemble(res.results), res


def kernel(**inputs) -> np.ndarray:
    out, _ = run(inputs, trace=False)
    return out


# revision 37
# speedup vs baseline: 1.2044x; 1.0188x over previous
"""Bass/Tile TRN2 kernel: 16-head MHA (B=2, T=2048, D=1024, H=64) on 8 NeuronCores.

Sharding: 8-way tensor parallel over heads — core c handles heads {2c, 2c+1}
for BOTH batches. After attention each (batch, tq-block) "block" s (8 total)
is re-sharded so that core c owns COLUMN SLICE c (64 rows) of every block:
one small AllToAll per block ([8*128, 64] f16, 128 KB) fires as soon as that
block is normalized on all cores, fully overlapped with the remaining
attention compute. The output projection runs per block-PAIR (two 64-row
slices stacked into a full 128-partition matmul), also overlapped; only the
last block's AllToAll + projection remain on the tail.

Per-core device pipeline (all FLOPs on device):
  - Activation loads are chunked (xv/xk by d-chunk, xq by (tq-block, d-chunk))
    so the V projection starts on chunk 0 instead of after the full 12 MB
    preamble; Q projection is interleaved per tq-block into the attention
    loop. b=1 activations stream during b=0 attention.
  - QKV projections as f16 matmuls accumulating fp32 in PSUM; activations
    arrive pre-transposed ([D, T]) so the contraction dim d sits on SBUF
    partitions. 1/sqrt(H) is folded into Wq/bq on host.
  - The attention steady state is ACT-bound (~1.12us per [128,1024] exp), so
    serial PE phases are interleaved into the blocks' PE slack as emission
    "pieces": the next block's Q projection (two 4-matmul halves mid-block)
    and ALL of b=1's V/K projection + V-transpose work (8 self-contained
    pieces in blocks (0,2)/(0,3)) — the b0->b1 transition is nearly empty.
  - Scores S^T[tk, tq] = K^T.T @ Q^T per head; the two heads are issued
    back-to-back as row-tiled (K=64, partitions 0-63 / 64-127) matmuls so they
    run concurrently on the PE array.
  - exp on ScalarE straight out of PSUM, f16 out, alternating with a
    DVE-copy path (ACT PSUM reads throttle concurrent PE matmuls ~1.8x).
  - PV matmul with a ones-augmented V (65 stationary columns) so row 64 of the
    PV accumulator is the softmax denominator for free.
  - Normalize with no DRAM bounces and no slow DVE reciprocal: denominator
    reciprocal as exp(-ln(x)) on ScalarE (Ln/Exp share one activation table),
    partition-broadcast via a K=1 ones-matmul on the PE, DVE multiply.
  - Per-block AllToAll (two dummy warmup collectives at t~0 absorb the
    ~10us-per-op CC-stream warmup), per-pair output projection, direct DMA of
    the [2*64, 512] result slabs to the output tensor.
  - Bulk prefetch (wp/bp, b=1 activations) drains 7 chunks per block end so
    latency-critical staging DMAs never queue behind megabytes on the rings;
    all 25 chunks MUST be emitted before the b=1 projection pieces read them.
Host does layout-only prep (transpose, f16 cast, weight slicing/replication)
and scatters the 8 cores' [8, 64, D] slabs into the full output.
"""

import os
import sys
from contextlib import ExitStack

import numpy as np

sys.path.insert(0, "/opt/trn_rl_repo")

import ml_dtypes  # noqa: E402

F16 = np.float16

B, T, D = 2, 2048, 1024
N_HEADS, H = 16, 64
NCORES = 8
GROUPS = [[0, 1, 2, 3, 4, 5, 6, 7]]
NLOC = 2            # heads per core
TQB = 512           # tq block size
NTQB = T // TQB     # 4
NBLK = B * NTQB     # 8 blocks total
TKC = 128           # tk chunk size
NTKC = T // TKC     # 16
DC = 128            # d chunk
NDC = D // DC       # 8
SLOTS = 2           # score-psum slots per exp tile ([128, 2*512] = 2 banks)
VA = 128            # V_aug stationary width: [V(64) | ones(1) | junk(63)]
NW = NLOC * H       # 128: projection output width per core
XO = 64             # output rows per core per block (column slice)
NHC = N_HEADS * H // 128  # 8 head-dim chunks in the output projection

INTERLEAVE_QPROJ = True
EXP_DIRECT = True

_CACHE = {}


def _legalize_waits(bir_bytes):
    """This toolchain's walrus accepts at most ONE semaphore wait per
    instruction ("Too many sync wait commands"). Tile's sem assignment emits
    several. Hoist all but one wait of each instruction onto same-engine NoOps
    inserted immediately before it (engines execute their stream in order, so
    waiting earlier on the same engine is equivalent)."""
    import json

    j = json.loads(bir_bytes)
    ctr = 0
    for fn in j["functions"]:
        for blk in fn["blocks"]:
            out = []
            for ins in blk["instructions"]:
                si = ins.get("sync_info")
                waits = (si or {}).get("on_wait") or []
                if len(waits) > 1:
                    for w in waits[:-1]:
                        ctr += 1
                        out.append(
                            {
                                "engine": ins["engine"],
                                "ins": [],
                                "outs": [],
                                "name": f"waitfix-{ctr}",
                                "opcode": "NoOp",
                                "sync_info": {"on_wait": [w], "on_update": []},
                            }
                        )
                    si["on_wait"] = [waits[-1]]
                out.append(ins)
            blk["instructions"] = out
    return json.dumps(j).encode()


def _build():
    import concourse.bass as bass
    import concourse.mybir as mybir
    import concourse.tile as tile

    f32 = mybir.dt.float32
    f16 = mybir.dt.float16
    AF = mybir.ActivationFunctionType
    ALU = mybir.AluOpType

    nc = bass.Bass(
        "TRN2", target_bir_lowering=False, debug=False, num_devices=NCORES
    )

    qT = [nc.dram_tensor(f"qT{b}", [D, T], f16, kind="ExternalInput") for b in range(B)]
    kT = [nc.dram_tensor(f"kT{b}", [D, T], f16, kind="ExternalInput") for b in range(B)]
    vT = [nc.dram_tensor(f"vT{b}", [D, T], f16, kind="ExternalInput") for b in range(B)]
    wq = nc.dram_tensor("wq", [D, NW], f16, kind="ExternalInput")
    wk = nc.dram_tensor("wk", [D, NW], f16, kind="ExternalInput")
    wv = nc.dram_tensor("wv", [D, NW], f16, kind="ExternalInput")
    wp = nc.dram_tensor("wp", [N_HEADS * H, D], f16, kind="ExternalInput")
    bq = nc.dram_tensor("bq", [128, 1], f32, kind="ExternalInput")
    bk = nc.dram_tensor("bk", [128, 1], f32, kind="ExternalInput")
    bv = nc.dram_tensor("bv", [128, 1], f32, kind="ExternalInput")
    bp = nc.dram_tensor("bp", [128, D], f32, kind="ExternalInput")
    ident = nc.dram_tensor("ident", [128, 128], f16, kind="ExternalInput")
    out = nc.dram_tensor("out", [NBLK, XO, D], f32, kind="ExternalOutput")

    with tile.TileContext(nc) as tc, ExitStack() as ctx:
        p_const = ctx.enter_context(tc.tile_pool(name="const", bufs=1))
        p_xt = ctx.enter_context(tc.tile_pool(name="xt", bufs=1))
        p_qk = ctx.enter_context(tc.tile_pool(name="qk", bufs=4))
        p_va = ctx.enter_context(tc.tile_pool(name="va", bufs=2))
        p_pt = ctx.enter_context(tc.tile_pool(name="pt", bufs=3))
        p_a = ctx.enter_context(tc.tile_pool(name="a", bufs=3))
        p_at = ctx.enter_context(tc.tile_pool(name="at", bufs=2))
        p_o = ctx.enter_context(tc.tile_pool(name="o", bufs=2))
        p_dram = ctx.enter_context(tc.tile_pool(name="dram", bufs=1, space="DRAM"))

        # ---- small constant loads (wp/bp deferred past the x0 loads) --------
        wq_sb = p_const.tile([128, NDC * NW], f16)
        wk_sb = p_const.tile([128, NDC * NW], f16)
        wv_sb = p_const.tile([128, NDC * NW], f16)
        bq_sb = p_const.tile([128, 1], f32)
        bk_sb = p_const.tile([128, 1], f32)
        bv_sb = p_const.tile([128, 1], f32)
        id_sb = p_const.tile([128, 128], f16)
        ones64 = p_const.tile([1, H], f16)
        nc.vector.memset(ones64[:], 1.0)
        nc.sync.dma_start(id_sb[:], ident[:])
        for sb_t, ext in ((wv_sb, wv), (wk_sb, wk), (wq_sb, wq)):
            nc.sync.dma_start(
                sb_t[:].rearrange("p (c m) -> p c m", m=NW),
                ext[:].rearrange("(c p) m -> p c m", p=128),
            )
        for sb_t, ext in ((bv_sb, bv), (bk_sb, bk), (bq_sb, bq)):
            nc.sync.dma_start(sb_t[:], ext[:])

        a2i = [
            p_dram.tile([NCORES * NW, XO], f16, name=f"a2i{s}", tag=f"a2i{s}")
            for s in range(NBLK)
        ]
        a2o = [
            p_dram.tile([NCORES * NW, XO], f16, name=f"a2o{s}", tag=f"a2o{s}")
            for s in range(NBLK)
        ]
        # two dummy collectives issued at t~0: the first two CC-stream ops pay
        # ~10us trigger-warmup each, so burn them during the DMA preamble
        wrm_i = p_dram.tile([NCORES, XO], f16, name="wrm_i", tag="wrm_i")
        wrm_o = p_dram.tile([NCORES, XO], f16, name="wrm_o", tag="wrm_o")
        wrm_o2 = p_dram.tile([NCORES, XO], f16, name="wrm_o2", tag="wrm_o2")

        ps_sc = ctx.enter_context(tc.tile_pool(name="ps_sc", bufs=3, space="PSUM"))
        ps_pv = ctx.enter_context(tc.tile_pool(name="ps_pv", bufs=2, space="PSUM"))

        def load_xvk(b, xk, xv):
            for sb_t, ext in ((xv, vT[b]), (xk, kT[b])):
                for dc in range(NDC):
                    nc.sync.dma_start(
                        sb_t[:, dc * T : (dc + 1) * T],
                        ext[dc * 128 : (dc + 1) * 128, :],
                    )

        def load_xq(b, xq):
            # chunked by (tq-block, d-chunk) so Q-proj(j) starts after 1 MB
            for j in range(NTQB):
                for dc in range(NDC):
                    nc.sync.dma_start(
                        xq[:, dc * T + j * TQB : dc * T + (j + 1) * TQB],
                        qT[b][dc * 128 : (dc + 1) * 128, j * TQB : (j + 1) * TQB],
                    )

        def mk_vk_pieces(b, xk, xv, st):
            """vk_proj split into 8 self-contained emitter pieces (each opens
            and closes its own PSUM group) so b=1's V/K projections can be
            interleaved into b=0's ACT-bound attention blocks."""

            def vtb(tb):
                if tb == 0:
                    st["vt"] = p_va.tile([128, T], f16, name=f"vt{b}", tag="vt")
                vt = st["vt"]
                psv = ps_sc.tile([128, TQB], f32, name=f"psv{b}{tb}", tag="sc")
                for dc in range(NDC):
                    nc.tensor.matmul(
                        psv[:],
                        lhsT=wv_sb[:, dc * NW : (dc + 1) * NW],
                        rhs=xv[:, dc * T + tb * TQB : dc * T + (tb + 1) * TQB],
                        start=(dc == 0),
                        stop=(dc == NDC - 1),
                    )
                nc.vector.tensor_scalar(
                    vt[:, tb * TQB : (tb + 1) * TQB],
                    psv[:],
                    bv_sb[:, 0:1],
                    None,
                    ALU.add,
                )

            def transp(i0):
                if i0 == 0:
                    st["va"] = p_va.tile(
                        [128, NTKC * NLOC * VA], f16, name=f"va{b}", tag="va"
                    )
                    nc.vector.memset(
                        st["va"][:].rearrange(
                            "p (i h x) -> p i h x", h=NLOC, x=VA
                        )[:, :, :, H : H + 1],
                        1.0,
                    )
                va = st["va"]
                for i in range(i0, i0 + NTKC // 2):
                    pst = ps_sc.tile([128, 128], f16, name=f"pst{b}{i}", tag="sc")
                    nc.tensor.transpose(
                        pst[:], st["vt"][:, i * TKC : (i + 1) * TKC], id_sb[:]
                    )
                    dst = va[:, i * NLOC * VA : (i + 1) * NLOC * VA].rearrange(
                        "p (h x) -> p h x", x=VA
                    )[:, :, 0:H]
                    nc.vector.tensor_copy(
                        dst, pst[:].rearrange("p (h x) -> p h x", x=H)
                    )

            def ktb(tb):
                if tb == 0:
                    st["kt"] = p_qk.tile([128, T], f16, name=f"kt{b}", tag="qk")
                kt = st["kt"]
                ps = ps_sc.tile([128, TQB], f32, name=f"psk{b}{tb}", tag="sc")
                for dc in range(NDC):
                    nc.tensor.matmul(
                        ps[:],
                        lhsT=wk_sb[:, dc * NW : (dc + 1) * NW],
                        rhs=xk[:, dc * T + tb * TQB : dc * T + (tb + 1) * TQB],
                        start=(dc == 0),
                        stop=(dc == NDC - 1),
                    )
                nc.vector.tensor_scalar(
                    kt[:, tb * TQB : (tb + 1) * TQB],
                    ps[:],
                    bk_sb[:, 0:1],
                    None,
                    ALU.add,
                )

            return [
                lambda: vtb(0),
                lambda: vtb(1),
                lambda: vtb(2),
                lambda: vtb(3),
                lambda: (transp(0), ktb(0)),
                lambda: (transp(8), ktb(1)),
                lambda: ktb(2),
                lambda: ktb(3),
            ]

        def vk_proj(b, xk, xv):
            st = {}
            for piece in mk_vk_pieces(b, xk, xv, st):
                piece()
            return st["kt"], st["va"]

        def q_proj(b, j, xq, qt, half=None, state={}):
            if half in (None, 0):
                state[(b, j)] = ps_sc.tile(
                    [128, TQB], f32, name=f"psq{b}{j}", tag="sc"
                )
            ps = state[(b, j)]
            dcs = range(NDC) if half is None else (
                range(NDC // 2) if half == 0 else range(NDC // 2, NDC)
            )
            for dc in dcs:
                nc.tensor.matmul(
                    ps[:],
                    lhsT=wq_sb[:, dc * NW : (dc + 1) * NW],
                    rhs=xq[:, dc * T + j * TQB : dc * T + (j + 1) * TQB],
                    start=(dc == 0),
                    stop=(dc == NDC - 1),
                )
            if half in (None, 1):
                nc.vector.tensor_scalar(
                    qt[:, j * TQB : (j + 1) * TQB],
                    ps[:],
                    bq_sb[:, 0:1],
                    None,
                    ALU.add,
                )
                state.pop((b, j))

        # Bulk prefetch DMAs drain one-per-round so the latency-critical
        # staging transfers never sit behind megabytes of prefetch on the
        # DMA rings.
        prefq = []
        peq = []

        def pump():
            if prefq:
                dst, src = prefq.pop(0)
                nc.sync.dma_start(dst, src)

        def stage_dma(s, hd, an):
            nc.sync.dma_start(
                a2i[s][:].rearrange("(d h p) x -> h p d x", h=NLOC, p=H)[hd],
                an[:].rearrange("p (d x) -> p d x", x=XO),
            )

        def emit_cc(s):
            nc.gpsimd.collective_compute(
                "AllToAll",
                ALU.bypass,
                replica_groups=GROUPS,
                ins=[a2i[s].opt()],
                outs=[a2o[s].opt()],
            )

        def norm_fast(b, j, pv):
            """Normalize with no DRAM bounces: reciprocal as exp(-ln(x)) on
            the Scalar engine (Ln and Exp share one activation table set, so
            no table reload), partition-broadcast via a K=1 ones matmul on the
            PE, DVE multiply. All sub-us compute-engine hops (~2.5us total),
            so the chain neither rides the congested DMA rings nor parks
            long-latency waits on the DVE queue ahead of the exp CASTs."""
            s = b * NTQB + j
            for hd in range(NLOC):
                a_sb = p_a.tile([H + 1, TQB], f32, name=f"fa{b}{j}{hd}", tag="a")
                nc.vector.tensor_copy(a_sb[:], pv[hd][0 : H + 1, :])
                t32 = p_a.tile([1, TQB], f32, name=f"ft{b}{j}{hd}", tag="t32")
                nc.scalar.activation(t32[:], a_sb[H : H + 1, :], AF.Ln)
                rcf = p_a.tile([1, TQB], f16, name=f"fr{b}{j}{hd}", tag="rcf")
                nc.scalar.activation(rcf[:], t32[:], AF.Exp, 0.0, -1.0)
                rep = ps_sc.tile([H, TQB], f32, name=f"frp{b}{j}{hd}", tag="sc")
                nc.tensor.matmul(
                    rep[:], lhsT=ones64[0:1, :], rhs=rcf[0:1, :],
                    start=True, stop=True,
                )
                an = p_a.tile([H, TQB], f16, name=f"fan{b}{j}{hd}", tag="an")
                nc.vector.tensor_tensor(an[:], a_sb[0:H, :], rep[:], ALU.mult)
                stage_dma(s, hd, an)
            emit_cc(s)
            # 7 pops x 4 b=0 blocks = 28 >= the 25 queued prefetch chunks:
            # everything must be emitted before vk_proj(1) reads xv1/xk1
            for _ in range(7):
                pump()

        def attn_block(b, j, qt, kt, va, next_qproj=None, allow_pe_pieces=False):
            pv = [
                ps_pv.tile([VA, TQB], f32, name=f"pv{b}{j}{hd}", tag="pv")
                for hd in range(NLOC)
            ]
            pv_emitted = [0, 0]
            # dual-rounds: tiles r and r+1 (i = r), PVs emitted in
            # reversed tile order so the second pair needs no new wait
            # (covered by the ACT-queue wait of the first pair).
            for r0 in range(0, NTKC, 2):
                pts = []
                for r in (r0, r0 + 1):
                    pss = ps_sc.tile(
                        [128, SLOTS * TQB], f32, name=f"pss{b}{j}{r}", tag="sc"
                    )
                    for hd in range(NLOC):
                        nc.tensor.matmul(
                            pss[:, hd * TQB : (hd + 1) * TQB],
                            lhsT=kt[
                                hd * H : (hd + 1) * H, r * TKC : (r + 1) * TKC
                            ],
                            rhs=qt[
                                hd * H : (hd + 1) * H, j * TQB : (j + 1) * TQB
                            ],
                            start=True,
                            stop=True,
                        )
                    # exp evacuation: ACT reading PSUM throttles
                    # concurrent PE matmuls ~1.8x, DVE PSUM reads do
                    # not — but the DVE fp32->f16 CAST is 1x-slow, so
                    # alternate the two paths.
                    pt = p_pt.tile(
                        [128, SLOTS * TQB], f16, name=f"pt{b}{j}{r}", tag="pt"
                    )
                    if EXP_DIRECT or r % 2 == 0:
                        nc.scalar.activation(pt[:], pss[:], AF.Exp)
                    else:
                        s_sb = p_pt.tile(
                            [128, SLOTS * TQB],
                            f16,
                            name=f"ss{b}{j}{r}",
                            tag="ss",
                        )
                        nc.vector.tensor_copy(s_sb[:], pss[:])
                        nc.scalar.activation(pt[:], s_sb[:], AF.Exp)
                    pts.append((r, pt))
                for r, pt in reversed(pts):
                    for hd in range(NLOC):
                        col0 = r * NLOC * VA + hd * VA
                        nc.tensor.matmul(
                            pv[hd][:],
                            lhsT=va[:, col0 : col0 + VA],
                            rhs=pt[:, hd * TQB : (hd + 1) * TQB],
                            start=(pv_emitted[hd] == 0),
                            stop=(pv_emitted[hd] == NTKC - 1),
                        )
                        pv_emitted[hd] += 1
                if r0 in (6, 8) and next_qproj is not None and INTERLEAVE_QPROJ:
                    # emit the NEXT block's Q projection mid-block (two 4-
                    # matmul halves): it runs in PE slack on a warm clock
                    # instead of serially at the (cold, ACT-idle) boundary.
                    # NOTE the psq tile holds an sc-ring slot from alloc to
                    # bias-read; only 2 pss allocations may intervene (ring=3)
                    next_qproj(0 if r0 == 6 else 1)
                if r0 in (4, 10, 12, 14) and allow_pe_pieces and peq:
                    # one self-contained b=1 V/K-projection piece per round:
                    # runs in PE slack instead of serially at the b0->b1
                    # transition
                    peq.pop(0)()
            norm_fast(b, j, pv)

        def outproj(P):
            # blocks 2P and 2P+1 stacked: full 128-partition matmul
            at = p_at.tile([128, NHC * 2 * XO], f16, name=f"at{P}", tag="at")
            for s01 in range(2):
                s = 2 * P + s01
                nc.sync.dma_start(
                    at[:].rearrange("p (c s x) -> p c s x", s=2, x=XO)[
                        :, :, s01, :
                    ],
                    a2o[s][:].rearrange("(c p) x -> p c x", p=128),
                )
            for dh in range(2):
                pso = ps_sc.tile([128, 512], f32, name=f"pso{P}{dh}", tag="sc")
                for c in range(NHC):
                    nc.tensor.matmul(
                        pso[:],
                        lhsT=at[:, c * 2 * XO : (c + 1) * 2 * XO],
                        rhs=wp_sb[:, c * D + dh * 512 : c * D + dh * 512 + 512],
                        start=(c == 0),
                        stop=(c == NHC - 1),
                    )
                o_sb = p_o.tile([128, 512], f32, name=f"o{P}{dh}", tag="o")
                nc.vector.tensor_tensor(
                    o_sb[:],
                    pso[:],
                    bp_sb[:, dh * 512 : (dh + 1) * 512],
                    ALU.add,
                )
                nc.sync.dma_start(
                    out[2 * P : 2 * P + 2, :, dh * 512 : (dh + 1) * 512]
                    .rearrange("s p x -> (s p) x"),
                    o_sb[:],
                )

        # ===== schedule ======================================================
        xts = {}
        for b in range(B):
            xts[b] = (
                p_xt.tile([128, NDC * T], f16, name=f"xq{b}", tag="xq"),
                p_xt.tile([128, NDC * T], f16, name=f"xk{b}", tag="xk"),
                p_xt.tile([128, NDC * T], f16, name=f"xv{b}", tag="xv"),
            )

        for wo in (wrm_o, wrm_o2):
            nc.gpsimd.collective_compute(
                "AllToAll",
                ALU.bypass,
                replica_groups=GROUPS,
                ins=[wrm_i.opt()],
                outs=[wo.opt()],
            )
        load_xvk(0, xts[0][1], xts[0][2])
        load_xq(0, xts[0][0])
        kt0, va0 = vk_proj(0, xts[0][1], xts[0][2])

        # Deferred bulk loads (wp/bp, b=1 K/V activations): drained one chunk
        # per dual-round by pump(), so the latency-critical normalize/staging
        # transfers never sit behind megabytes of prefetch on the DMA rings.
        wp_sb = p_const.tile([128, NHC * D], f16, name="wp_sb")
        bp_sb = p_const.tile([128, D], f32, name="bp_sb")
        for sb_t, ext in ((xts[1][2], vT[1]), (xts[1][1], kT[1])):
            for dc in range(NDC):
                prefq.append(
                    (
                        sb_t[:, dc * T : (dc + 1) * T],
                        ext[dc * 128 : (dc + 1) * 128, :],
                    )
                )
        for c in range(NHC):
            prefq.append(
                (wp_sb[:, c * D : (c + 1) * D], wp[c * 128 : (c + 1) * 128, :])
            )
        prefq.append((bp_sb[:], bp[:]))

        # b=1 V/K projection pieces, interleaved into b0 blocks (0,2)/(0,3)
        st1 = {}
        peq.extend(mk_vk_pieces(1, xts[1][1], xts[1][2], st1))

        qt0 = p_qk.tile([128, T], f16, name="qt0", tag="qk")
        q_proj(0, 0, xts[0][0], qt0)
        for j in range(NTQB):
            nq = (
                (lambda h, jn=j + 1: q_proj(0, jn, xts[0][0], qt0, half=h))
                if j + 1 < NTQB
                else None
            )
            attn_block(0, j, qt0, kt0, va0, next_qproj=nq,
                       allow_pe_pieces=(j >= 2))
            if not INTERLEAVE_QPROJ and j + 1 < NTQB:
                q_proj(0, j + 1, xts[0][0], qt0)
        while peq:
            peq.pop(0)()
        outproj(0)
        # xq0's slot is fully read (q_proj(0,3) issued above)
        load_xq(1, xts[1][0])

        kt1, va1 = st1["kt"], st1["va"]
        qt1 = p_qk.tile([128, T], f16, name="qt1", tag="qk")
        q_proj(1, 0, xts[1][0], qt1)
        for j in range(NTQB):
            nq = (
                (lambda h, jn=j + 1: q_proj(1, jn, xts[1][0], qt1, half=h))
                if j + 1 < NTQB
                else None
            )
            attn_block(1, j, qt1, kt1, va1, next_qproj=nq)
            if not INTERLEAVE_QPROJ and j + 1 < NTQB:
                q_proj(1, j + 1, xts[1][0], qt1)
            if j == 1:
                outproj(1)
        outproj(2)
        outproj(3)

    orig_to_json = nc.to_json_bytes
    nc.to_json_bytes = lambda: _legalize_waits(orig_to_json())
    return nc


def _get_nc():
    if "nc" not in _CACHE:
        _CACHE["nc"] = _build()
    return _CACHE["nc"]


def _make_in_maps(inputs):
    q = np.asarray(inputs["q"], dtype=np.float32)
    v = np.asarray(inputs["v"], dtype=np.float32)
    k = np.asarray(inputs["k"], dtype=np.float32)
    w_query = np.asarray(inputs["w_query"], dtype=np.float32)
    b_query = np.asarray(inputs["b_query"], dtype=np.float32)
    w_value = np.asarray(inputs["w_value"], dtype=np.float32)
    b_value = np.asarray(inputs["b_value"], dtype=np.float32)
    w_key = np.asarray(inputs["w_key"], dtype=np.float32)
    b_key = np.asarray(inputs["b_key"], dtype=np.float32)
    w_projection = np.asarray(inputs["w_projection"], dtype=np.float32)
    b_projection = np.asarray(inputs["b_projection"], dtype=np.float32)

    scale = np.float32(1.0 / np.sqrt(H))
    wp_s = np.ascontiguousarray(
        w_projection.transpose(0, 2, 1).reshape(N_HEADS * H, D)
    ).astype(F16)
    bp_s = np.ascontiguousarray(
        np.tile(b_projection.reshape(1, D), (128, 1))
    ).astype(np.float32)

    xT = {}
    for b in range(B):
        xT[b] = tuple(
            np.ascontiguousarray(x[b].T).astype(F16) for x in (q, k, v)
        )

    in_maps = []
    for c in range(NCORES):
        hs = c * NLOC
        wq_s = (w_query[:, hs : hs + NLOC, :].reshape(D, NW) * scale).astype(F16)
        wk_s = w_key[:, hs : hs + NLOC, :].reshape(D, NW).astype(F16)
        wv_s = w_value[:, hs : hs + NLOC, :].reshape(D, NW).astype(F16)
        bq_s = np.ascontiguousarray(
            (b_query[hs : hs + NLOC].reshape(NW) * scale).reshape(NW, 1)
        ).astype(np.float32)
        bk_s = np.ascontiguousarray(
            b_key[hs : hs + NLOC].reshape(NW, 1)
        ).astype(np.float32)
        bv_s = np.ascontiguousarray(
            b_value[hs : hs + NLOC].reshape(NW, 1)
        ).astype(np.float32)
        m = {
            "ident": np.eye(128, dtype=np.float32).astype(F16),
            "wq": np.ascontiguousarray(wq_s),
            "wk": np.ascontiguousarray(wk_s),
            "wv": np.ascontiguousarray(wv_s),
            "wp": wp_s,
            "bq": bq_s,
            "bk": bk_s,
            "bv": bv_s,
            "bp": bp_s,
        }
        for b in range(B):
            m[f"qT{b}"], m[f"kT{b}"], m[f"vT{b}"] = xT[b]
        in_maps.append(m)
    return in_maps


def _assemble(results):
    out = np.empty((B, T, D), np.float32)
    for c in range(NCORES):
        r = results[c]["out"]  # [NBLK, XO, D]
        for s in range(NBLK):
            b, j = divmod(s, NTQB)
            r0 = j * TQB + c * XO
            out[b, r0 : r0 + XO, :] = r[s]
    return out


def run(inputs, trace=False, **kwargs):
    from concourse.bass_utils import run_bass_kernel_spmd

    nc = _get_nc()
    in_maps = _make_in_maps(inputs)
    res = run_bass_kernel_spmd(
        nc, in_maps, list(range(NCORES)), trace=trace, **kwargs
    )
    return _assemble(res.results), res


def kernel(**inputs) -> np.ndarray:
    out, _ = run(inputs, trace=False)
    return out
